# revision 1
# baseline (speedup 1.0000x reference)
"""Trainium2 Bass kernel for CausalSelfAttention with sliding-window + sink mask.

Sharding: 8 cores = (batch 2) x (sequence chunks of 512). Each core computes
QKV (+RoPE) for its 512 queries and for a kv range [4 sink | 256 halo |
512 own | 124 zero-pad] = 896 positions, runs banded attention in a
scores-transposed [k, q] layout (7 key-chunks of 128 with fixed q-windows,
multiplicative post-exp masking, denominator via a ones-column in V), then
projects with w_proj emitting a transposed [C, 512] output that the host
re-transposes and concatenates.

All matmuls run as float32r (full-rate fp32 path of the PE).
"""

import numpy as np

B, T, C, NH, HD = 2, 2048, 1024, 16, 64
WIN, SINK = 256, 4
CH = 512          # queries per core
KV = 896          # 512 own + 256 halo + 4 sink + 124 pad
NCORES = 8
W_C = [384, 512, 256, 256, 128, 256, 512]
OFF_C = [0, 0, 256, 256, 0, 0, 0]
MOFF = np.concatenate([[0], np.cumsum(W_C)]).astype(int)  # mask col offsets
MTOT = int(MOFF[-1])  # 1924

_cache = {}


def _build_nc():
    import concourse.bacc as bacc
    import concourse.mybir as mybir
    import concourse.tile as tile

    f32 = mybir.dt.float32
    f32r = mybir.dt.float32r
    AF = mybir.ActivationFunctionType

    nc = bacc.Bacc("TRN2", target_bir_lowering=False, debug=False,
                   num_devices=NCORES)

    xT = nc.dram_tensor("xT", [C, KV], f32r, kind="ExternalInput").ap()
    wqs = nc.dram_tensor("wqs", [C, C], f32r, kind="ExternalInput").ap()
    wks = nc.dram_tensor("wks", [C, C], f32r, kind="ExternalInput").ap()
    wv = nc.dram_tensor("wv", [C, C], f32r, kind="ExternalInput").ap()
    wps = nc.dram_tensor("wps", [C, C], f32r, kind="ExternalInput").ap()
    cos_q = nc.dram_tensor("cos_q", [128, CH], f32, kind="ExternalInput").ap()
    sin_q = nc.dram_tensor("sin_q", [128, CH], f32, kind="ExternalInput").ap()
    cos_k = nc.dram_tensor("cos_k", [128, KV], f32, kind="ExternalInput").ap()
    sin_k = nc.dram_tensor("sin_k", [128, KV], f32, kind="ExternalInput").ap()
    masks = nc.dram_tensor("masks", [128, MTOT], f32, kind="ExternalInput").ap()
    p2d = nc.dram_tensor("p2", [128, 128], f32r, kind="ExternalInput").ap()
    rseld = nc.dram_tensor("rsel", [16, C], f32r, kind="ExternalInput").ap()
    onesd = nc.dram_tensor("ones", [128, 16], f32, kind="ExternalInput").ap()
    outT = nc.dram_tensor("outT", [C, CH], f32, kind="ExternalOutput").ap()

    KSEG = [(0, 512), (512, 384)]  # kv free-dim segments (psum bank limit)

    with tile.TileContext(nc) as tc:
        with (
            tc.tile_pool(name="pers", bufs=1) as pers,
            tc.tile_pool(name="wsl", bufs=2) as wsl,
            tc.tile_pool(name="big", bufs=8) as big,     # wv chunks then praw/outT
            tc.tile_pool(name="qk", bufs=2) as qkp,
            tc.tile_pool(name="tmp", bufs=2) as tmp,
            tc.tile_pool(name="yts", bufs=1) as ytsp,
            tc.tile_pool(name="ptp", bufs=14) as ptp,
            tc.tile_pool(name="sm", bufs=2) as smp,
            tc.tile_pool(name="psmm", bufs=2, space="PSUM") as psmm,
            tc.tile_pool(name="pssc", bufs=4, space="PSUM") as pssc,
            tc.tile_pool(name="psyt", bufs=2, space="PSUM") as psyt,
        ):
            # ---------- persistent loads ----------
            xa, xb = [], []
            for i in range(8):
                t = pers.tile([128, 512], f32r, tag=f"xa{i}", name=f"xa{i}")
                nc.sync.dma_start(t[:], xT[i * 128:(i + 1) * 128, 0:512])
                xa.append(t)
                t = pers.tile([128, 384], f32r, tag=f"xb{i}", name=f"xb{i}")
                nc.sync.dma_start(t[:], xT[i * 128:(i + 1) * 128, 512:896])
                xb.append(t)
            tcos_q = pers.tile([128, CH], f32, tag="cos_q")
            nc.sync.dma_start(tcos_q[:], cos_q[:])
            tsin_q = pers.tile([128, CH], f32, tag="sin_q")
            nc.sync.dma_start(tsin_q[:], sin_q[:])
            tcos_k = pers.tile([128, KV], f32, tag="cos_k")
            nc.sync.dma_start(tcos_k[:], cos_k[:])
            tsin_k = pers.tile([128, KV], f32, tag="sin_k")
            nc.sync.dma_start(tsin_k[:], sin_k[:])
            tmask = pers.tile([128, MTOT], f32, tag="mask")
            nc.sync.dma_start(tmask[:], masks[:])
            tp2 = pers.tile([128, 128], f32r, tag="p2")
            nc.sync.dma_start(tp2[:], p2d[:])
            trsel = pers.tile([16, C], f32r, tag="rsel")
            nc.sync.dma_start(trsel[:], rseld[:])
            tones = pers.tile([128, 16], f32, tag="ones")
            nc.sync.dma_start(tones[:], onesd[:])

            # ---------- V = xT.T @ wv in [k, d] layout with ones columns ----------
            wvc = []
            for kc in range(8):
                t = big.tile([128, 1024], f32r, tag="big", name=f"wvc{kc}")
                nc.sync.dma_start(t[:], wv[kc * 128:(kc + 1) * 128, :])
                wvc.append(t)
            v_sb = []
            for tt in range(7):
                vt = pers.tile([128, 1040], f32r, tag=f"v{tt}", name=f"v{tt}")
                vr = vt.rearrange("p (h e) -> p h e", e=65)
                pv = [psmm.tile([128, 512], f32, tag="mm", name=f"pv{tt}_{i}")
                      for i in range(2)]
                for kc in range(8):
                    xsl = (xa[kc][:, tt * 128:(tt + 1) * 128] if tt < 4 else
                           xb[kc][:, (tt - 4) * 128:(tt - 3) * 128])
                    for dh in range(2):
                        nc.tensor.matmul(
                            pv[dh][:], xsl,
                            wvc[kc][:, dh * 512:(dh + 1) * 512],
                            start=(kc == 0), stop=(kc == 7),
                        )
                for dh in range(2):
                    nc.scalar.copy(
                        vr[:, dh * 8:(dh + 1) * 8, 0:64],
                        pv[dh][:].rearrange("p (h e) -> p h e", e=64),
                    )
                nc.scalar.copy(vr[:, :, 64:65],
                               tones[:].rearrange("p (h o) -> p h o", o=1))
                v_sb.append(vt)

            d16 = smp.tile([16, 512], f32, tag="d16")
            AVORD = [1, 6, 0, 5, 4, 2, 3]

            def qkv_rope(hp):
                # qT raw
                wq_sl = wsl.tile([128, 1024], f32r, tag="wslab",
                                 name=f"wq{hp}")
                nc.sync.dma_start(wq_sl[:], wqs[hp * 128:(hp + 1) * 128, :])
                pq = psmm.tile([128, 512], f32, tag="mm", name=f"pq{hp}")
                for kc in range(8):
                    nc.tensor.matmul(
                        pq[:], wq_sl[:, kc * 128:(kc + 1) * 128],
                        xa[kc][:],
                        start=(kc == 0), stop=(kc == 7),
                    )
                qraw = tmp.tile([128, CH], f32r, tag="qraw", name=f"qraw{hp}")
                nc.scalar.copy(qraw[:], pq[:])

                # kT raw (segments share each weight load)
                wk_sl = wsl.tile([128, 1024], f32r, tag="wslab",
                                 name=f"wk{hp}")
                nc.sync.dma_start(wk_sl[:], wks[hp * 128:(hp + 1) * 128, :])
                kraw = tmp.tile([128, KV], f32r, tag="kraw", name=f"kraw{hp}")
                pk = [psmm.tile([128, 512], f32, tag="mm", name=f"pk{hp}_{i}")
                      for i in range(2)]
                for kc in range(8):
                    for si, (s0, sw) in enumerate(KSEG):
                        rhs = xa[kc][:] if si == 0 else xb[kc][:]
                        nc.tensor.matmul(
                            pk[si][:, 0:sw], wk_sl[:, kc * 128:(kc + 1) * 128],
                            rhs, start=(kc == 0), stop=(kc == 7),
                        )
                for si, (s0, sw) in enumerate(KSEG):
                    nc.scalar.copy(kraw[:, s0:s0 + sw], pk[si][:, 0:sw])

                # rope
                qT = qkp.tile([128, CH], f32r, tag="qT", name=f"qT{hp}")
                prot = psmm.tile([128, 512], f32, tag="mm", name=f"prot{hp}")
                nc.tensor.matmul(prot[:], tp2[:], qraw[:], start=True, stop=True)
                t2 = tmp.tile([128, CH], f32, tag="t2", name=f"t2q{hp}")
                nc.vector.tensor_mul(t2[:], prot[:], tsin_q[:])
                nc.vector.tensor_mul(qraw[:], qraw[:], tcos_q[:])
                nc.vector.tensor_add(qT[:], qraw[:], t2[:])

                kT = qkp.tile([128, KV], f32r, tag="kT", name=f"kT{hp}")
                for si, (s0, sw) in enumerate(KSEG):
                    prk = psmm.tile([128, 512], f32, tag="mm",
                                    name=f"prk{hp}_{si}")
                    nc.tensor.matmul(prk[:, 0:sw], tp2[:],
                                     kraw[:, s0:s0 + sw], start=True, stop=True)
                    t2k = tmp.tile([128, 512], f32, tag="t2",
                                   name=f"t2k{hp}_{si}")
                    nc.vector.tensor_mul(t2k[:, 0:sw], prk[:, 0:sw],
                                         tsin_k[:, s0:s0 + sw])
                    nc.vector.tensor_mul(kraw[:, s0:s0 + sw],
                                         kraw[:, s0:s0 + sw],
                                         tcos_k[:, s0:s0 + sw])
                    nc.vector.tensor_add(kT[:, s0:s0 + sw],
                                         kraw[:, s0:s0 + sw], t2k[:, 0:sw])
                return qT, kT

            def sc_block(hp, qT, kT):
                # scoresT matmuls issued as adjacent row-tile pairs (K=64 at
                # partition bases 0/64 -> concurrent in the PE array), then
                # exp (psum->sbuf, fused 1/sqrt(hd) scale) and mask multiply.
                pts = {}
                for c in range(7):
                    w, off = W_C[c], OFF_C[c]
                    scs = []
                    for half in range(2):
                        dsl = slice(half * 64, half * 64 + 64)
                        sc = pssc.tile([128, 512], f32, tag="sc",
                                       name=f"sc{hp}_{c}_{half}")
                        nc.tensor.matmul(
                            sc[:, 0:w], kT[dsl, c * 128:(c + 1) * 128],
                            qT[dsl, off:off + w], start=True, stop=True,
                        )
                        scs.append(sc)
                    for half in range(2):
                        w, off = W_C[c], OFF_C[c]
                        praw = big.tile([128, 512], f32r, tag="big",
                                        name=f"praw{hp}_{c}_{half}")
                        nc.scalar.activation(praw[:, 0:w], scs[half][:, 0:w],
                                             AF.Exp, scale=0.125)
                        pt = ptp.tile([128, 512], f32r, tag="pt",
                                      name=f"pt{hp}_{c}_{half}")
                        nc.vector.tensor_mul(
                            pt[:, 0:w], praw[:, 0:w],
                            tmask[:, MOFF[c]:MOFF[c] + w],
                        )
                        pts[(c, half)] = pt
                return pts

            def av_block(hp, pts):
                yt_pair = []
                for half in range(2):
                    h = hp * 2 + half
                    yt = psyt.tile([65, 512], f32, tag="yt",
                                   name=f"yt{hp}_{half}")
                    for ci, c in enumerate(AVORD):
                        w, off = W_C[c], OFF_C[c]
                        nc.tensor.matmul(
                            yt[:, off:off + w],
                            v_sb[c][:, h * 65:(h + 1) * 65],
                            pts[(c, half)][:, 0:w],
                            start=(ci == 0), stop=(ci == 6),
                        )
                    yt_pair.append(yt)
                ytu = ytsp.tile([128, CH], f32r, tag=f"ytu{hp}",
                                name=f"ytu{hp}")
                nc.scalar.copy(ytu[0:64, :], yt_pair[0][0:64, :])
                nc.scalar.copy(ytu[64:128, :], yt_pair[1][0:64, :])
                for half in range(2):
                    dt_ = smp.tile([1, 512], f32, tag="dt",
                                   name=f"dt{hp}_{half}")
                    nc.scalar.copy(dt_[:], yt_pair[half][64:65, :])
                    nc.sync.dma_start(
                        d16[2 * hp + half:2 * hp + half + 1, :], dt_[:])
                return ytu

            # ---------- software-pipelined head-pair loop ----------
            yts = []
            qk_state = qkv_rope(0)
            for hp in range(8):
                pts = sc_block(hp, *qk_state)
                if hp < 7:
                    qk_state = qkv_rope(hp + 1)
                yts.append(av_block(hp, pts))

            # ---------- batched normalization (in place, rounds to f32r) ----
            r16 = smp.tile([16, 512], f32r, tag="r16")
            with nc.allow_low_precision(reason="f32r recip for PE broadcast"):
                nc.vector.reciprocal(r16[:], d16[:])
            for hp in range(8):
                prb = psmm.tile([128, 512], f32, tag="mm", name=f"prb{hp}")
                nc.tensor.matmul(prb[:], trsel[:, hp * 128:(hp + 1) * 128],
                                 r16[:], start=True, stop=True)
                nc.vector.tensor_mul(yts[hp][0:64, :], yts[hp][0:64, :],
                                     prb[0:64, :])
                nc.vector.tensor_mul(yts[hp][64:128, :], yts[hp][64:128, :],
                                     prb[64:128, :])

            # ---------- projection (transposed output) ----------
            for cc in range(8):
                wp_sl = wsl.tile([128, 1024], f32r, tag="wslab",
                                 name=f"wp{cc}")
                nc.sync.dma_start(wp_sl[:], wps[cc * 128:(cc + 1) * 128, :])
                po = psmm.tile([128, 512], f32, tag="mm", name=f"po{cc}")
                for hp in range(8):
                    nc.tensor.matmul(
                        po[:], wp_sl[:, hp * 128:(hp + 1) * 128], yts[hp][:],
                        start=(hp == 0), stop=(hp == 7),
                    )
                osb = big.tile([128, 512], f32, tag="big", name=f"osb{cc}")
                nc.scalar.copy(osb[:], po[:])
                nc.sync.dma_start(outT[cc * 128:(cc + 1) * 128, :], osb[:])

    nc.compile()
    return nc


def _host_inputs(x, w_attn, w_proj):
    """Build the 8 per-core input maps."""
    inv_freq = 1.0 / (10000.0 ** (np.arange(0, HD, 2, dtype=np.float32) / HD))
    iff = np.concatenate([inv_freq, inv_freq])  # [64]

    def cos_sin(pos):
        ang = pos[None, :].astype(np.float32) * iff[:, None]
        c = np.concatenate([np.cos(ang), np.cos(ang)], 0).astype(np.float32)
        s = np.concatenate([np.sin(ang), np.sin(ang)], 0).astype(np.float32)
        return np.ascontiguousarray(c), np.ascontiguousarray(s)

    P2 = np.zeros((128, 128), np.float32)
    for blk in range(2):
        o = blk * 64
        for d in range(32):
            P2[o + d + 32, o + d] = -1.0
            P2[o + d, o + d + 32] = 1.0

    rsel = np.zeros((16, C), np.float32)
    for h in range(16):
        hp, half = h // 2, h % 2
        rsel[h, hp * 128 + half * 64: hp * 128 + half * 64 + 64] = 1.0
    ones16 = np.ones((128, 16), np.float32)

    def shuffle_lhsT(w):
        # rows (kc*128 + c_lo), cols (hp*128 + d) ->
        # rows (hp*128 + c_lo), cols (kc*128 + d)
        return np.ascontiguousarray(
            w.reshape(8, 128, 8, 128).transpose(2, 1, 0, 3).reshape(C, C)
        )

    wq = shuffle_lhsT(w_attn[:, 0:C])
    wk = shuffle_lhsT(w_attn[:, C:2 * C])
    wvm = np.ascontiguousarray(w_attn[:, 2 * C:3 * C])
    wp = shuffle_lhsT(w_proj)

    in_maps = []
    for core in range(NCORES):
        b, j = core // 4, core % 4
        q0 = j * CH
        kv_gk = np.full(KV, -1, np.int64)
        kv_gk[0:512] = q0 + np.arange(CH)
        halo = q0 - 256 + np.arange(256)
        kv_gk[512:768] = np.where(halo >= 0, halo, -1)
        kv_gk[768:772] = np.arange(4)

        xTc = np.zeros((C, KV), np.float32)
        valid = kv_gk >= 0
        xTc[:, valid] = x[b, kv_gk[valid]].T

        cq, sq = cos_sin(q0 + np.arange(CH))
        ck, sk = cos_sin(np.maximum(kv_gk, 0))

        gq = q0 + np.arange(CH)
        mask = np.zeros((128, MTOT), np.float32)
        for c in range(7):
            rows = c * 128 + np.arange(128)
            gk = kv_gk[rows]
            qw = gq[OFF_C[c]:OFF_C[c] + W_C[c]]
            real = (rows < 772) & (gk >= 0)
            g = np.where(real, gk, 0)[:, None]
            qq = qw[None, :]
            is_sink = ((rows >= 768) & (rows < 772))[:, None]
            allow = np.where(
                is_sink,
                (g <= qq) & (qq - g >= WIN),
                (g <= qq) & (qq - g < WIN),
            )
            allow &= real[:, None]
            mask[:, MOFF[c]:MOFF[c] + W_C[c]] = allow.astype(np.float32)

        in_maps.append({
            "xT": xTc, "wqs": wq, "wks": wk, "wv": wvm, "wps": wp,
            "cos_q": cq, "sin_q": sq, "cos_k": ck, "sin_k": sk,
            "masks": mask, "p2": P2, "rsel": rsel, "ones": ones16,
        })
    return in_maps


def kernel(x, w_attn, w_proj):
    from concourse import bass_utils

    x = np.asarray(x, np.float32)
    w_attn = np.asarray(w_attn, np.float32)
    w_proj = np.asarray(w_proj, np.float32)

    if "nc" not in _cache:
        _cache["nc"] = _build_nc()
    nc = _cache["nc"]

    in_maps = _host_inputs(x, w_attn, w_proj)
    res = bass_utils.run_bass_kernel_spmd(nc, in_maps, list(range(NCORES)),
                                          **_cache.get("run_kwargs", {}))
    _cache["last_result"] = res

    y = np.zeros((B, T, C), np.float32)
    for core in range(NCORES):
        b, j = core // 4, core % 4
        y[b, j * CH:(j + 1) * CH, :] = res.results[core]["outT"].T
    return y



# revision 4
# speedup vs baseline: 1.1878x; 1.1878x over previous
"""Trainium2 Bass kernel for CausalSelfAttention with sliding-window + sink mask.

Sharding: 8 cores = (batch 2) x (sequence chunks of 512). Each core computes
QKV (+RoPE) for its 512 queries and a kv range [512 own | 256 halo] = 768
positions (6 chunks of 128); the 4 attention-sink K/V rows are computed on
the host and uploaded as tiny persistent tiles. Banded attention runs in a
scores-transposed [kv, q] layout with per-chunk q-windows, exp on the scalar
engine, multiplicative 0/1 masking split across vector+gpsimd, denominator
via a ones-column in V, per-head-pair normalization inside the loop (PE
broadcast of a 2-row reciprocal), then a preloaded-weight projection that
emits a transposed [C, 512] output the host re-transposes and concatenates.

All matmul operands are bf16 (full-rate PE path, no sub-256 f32r penalty),
accumulation stays f32 in PSUM.
"""

import numpy as np

B, T, C, NH, HD = 2, 2048, 1024, 16, 64
WIN, SINK = 256, 4
CH = 512          # queries per core
KV = 768          # 512 own + 256 halo (sink handled separately)
NCORES = 8
W_C = [384, 384, 256, 128, 128, 256, 512]
OFF_C = [0, 128, 256, 384, 0, 0, 0]
MOFF = np.concatenate([[0], np.cumsum(W_C)]).astype(int)
MTOT = int(MOFF[-1])  # 2048

_cache = {}


def _build_nc():
    import concourse.bacc as bacc
    import concourse.mybir as mybir
    import concourse.tile as tile

    f32 = mybir.dt.float32
    f32r = mybir.dt.float32r
    bf16 = mybir.dt.bfloat16
    AF = mybir.ActivationFunctionType

    nc = bacc.Bacc("TRN2", target_bir_lowering=False, debug=False,
                   num_devices=NCORES)

    xT = nc.dram_tensor("xT", [C, KV], bf16, kind="ExternalInput").ap()
    wqs = nc.dram_tensor("wqs", [C, C], bf16, kind="ExternalInput").ap()
    wks = nc.dram_tensor("wks", [C, C], bf16, kind="ExternalInput").ap()
    wv = nc.dram_tensor("wv", [C, C], bf16, kind="ExternalInput").ap()
    wps = nc.dram_tensor("wps", [C, C], bf16, kind="ExternalInput").ap()
    cos_q = nc.dram_tensor("cos_q", [128, CH], bf16, kind="ExternalInput").ap()
    sin_q = nc.dram_tensor("sin_q", [128, CH], bf16, kind="ExternalInput").ap()
    cos_k = nc.dram_tensor("cos_k", [128, KV], bf16, kind="ExternalInput").ap()
    sin_k = nc.dram_tensor("sin_k", [128, KV], bf16, kind="ExternalInput").ap()
    masks = nc.dram_tensor("masks", [128, MTOT], bf16,
                           kind="ExternalInput").ap()
    p2d = nc.dram_tensor("p2", [128, 128], bf16, kind="ExternalInput").ap()
    vsinkd = nc.dram_tensor("vsink", [4, 1040], bf16,
                            kind="ExternalInput").ap()
    ksinkd = nc.dram_tensor("ksink", [128, 32], bf16,
                            kind="ExternalInput").ap()
    sel2d = nc.dram_tensor("sel2", [2, 128], f32r, kind="ExternalInput").ap()
    outT = nc.dram_tensor("outT", [C, CH], f32, kind="ExternalOutput").ap()

    KSEG = [(0, 512), (512, 256)]  # kv free-dim segments (psum bank limit)
    SCORD = [6, 0, 1, 2, 3, 4, 5]  # sink first: AV accumulation starts full

    with tile.TileContext(nc) as tc:
        with (
            nc.allow_low_precision(reason="bf16 matmul operands throughout"),
            tc.tile_pool(name="pers", bufs=1) as pers,
            tc.tile_pool(name="wsl", bufs=2) as wsl,
            tc.tile_pool(name="big", bufs=8) as big,     # wv chunks then praw
            tc.tile_pool(name="qk", bufs=2) as qkp,
            tc.tile_pool(name="tmp", bufs=2) as tmp,
            tc.tile_pool(name="yts", bufs=1) as ytsp,
            tc.tile_pool(name="ptp", bufs=10) as ptp,
            tc.tile_pool(name="sm", bufs=4) as smp,
            tc.tile_pool(name="psmm", bufs=2, space="PSUM") as psmm,
            tc.tile_pool(name="pssc", bufs=3, space="PSUM") as pssc,
            tc.tile_pool(name="psyt", bufs=2, space="PSUM") as psyt,
            tc.tile_pool(name="psnb", bufs=1, space="PSUM") as psnb,
        ):
            # ---------- persistent loads (ordered for fast PE start) ------
            xa, xb = [], []
            for i in range(8):
                t = pers.tile([128, 512], bf16, tag=f"xa{i}", name=f"xa{i}")
                nc.sync.dma_start(t[:], xT[i * 128:(i + 1) * 128, 0:512])
                xa.append(t)
            for i in range(8):
                t = pers.tile([128, 256], bf16, tag=f"xb{i}", name=f"xb{i}")
                nc.sync.dma_start(t[:], xT[i * 128:(i + 1) * 128, 512:768])
                xb.append(t)
            tp2 = pers.tile([128, 128], bf16, tag="p2")
            nc.sync.dma_start(tp2[:], p2d[:])
            tcos_q = pers.tile([128, CH], bf16, tag="cos_q")
            nc.sync.dma_start(tcos_q[:], cos_q[:])
            tsin_q = pers.tile([128, CH], bf16, tag="sin_q")
            nc.sync.dma_start(tsin_q[:], sin_q[:])
            tcos_k = pers.tile([128, KV], bf16, tag="cos_k")
            nc.sync.dma_start(tcos_k[:], cos_k[:])
            tsin_k = pers.tile([128, KV], bf16, tag="sin_k")
            nc.sync.dma_start(tsin_k[:], sin_k[:])
            tksink = pers.tile([128, 32], bf16, tag="ksink")
            nc.sync.dma_start(tksink[:], ksinkd[:])
            tsel = []
            for half in range(2):
                t = pers.tile([1, 128], f32r, tag=f"sel{half}")
                nc.sync.dma_start(t[:], sel2d[half:half + 1, :])
                tsel.append(t)

            # ---------- qkv + rope (emitted per head-pair) ----------------
            def qkv_rope(hp):
                wq_sl = wsl.tile([128, 1024], bf16, tag="wslab",
                                 name=f"wq{hp}")
                nc.sync.dma_start(wq_sl[:], wqs[hp * 128:(hp + 1) * 128, :])
                pq = psmm.tile([128, 512], f32, tag="mm", name=f"pq{hp}")
                for kc in range(8):
                    nc.tensor.matmul(
                        pq[:], wq_sl[:, kc * 128:(kc + 1) * 128],
                        xa[kc][:],
                        start=(kc == 0), stop=(kc == 7),
                    )
                qraw = tmp.tile([128, CH], bf16, tag="qraw", name=f"qraw{hp}")
                nc.scalar.copy(qraw[:], pq[:])

                wk_sl = wsl.tile([128, 1024], bf16, tag="wslab",
                                 name=f"wk{hp}")
                nc.sync.dma_start(wk_sl[:], wks[hp * 128:(hp + 1) * 128, :])
                kraw = tmp.tile([128, KV], bf16, tag="kraw", name=f"kraw{hp}")
                pk = [psmm.tile([128, 512], f32, tag="mm", name=f"pk{hp}_{i}")
                      for i in range(2)]
                for kc in range(8):
                    for si, (s0, sw) in enumerate(KSEG):
                        rhs = xa[kc][:] if si == 0 else xb[kc][:]
                        nc.tensor.matmul(
                            pk[si][:, 0:sw], wk_sl[:, kc * 128:(kc + 1) * 128],
                            rhs, start=(kc == 0), stop=(kc == 7),
                        )
                for si, (s0, sw) in enumerate(KSEG):
                    nc.scalar.copy(kraw[:, s0:s0 + sw], pk[si][:, 0:sw])

                # rope: out = raw*cos + (P2@raw)*sin ; all-bf16 muls get DVE 2x
                qT = qkp.tile([128, CH], bf16, tag="qT", name=f"qT{hp}")
                prot = psmm.tile([128, 512], f32, tag="mm", name=f"prot{hp}")
                nc.tensor.matmul(prot[:], tp2[:], qraw[:], start=True,
                                 stop=True)
                t2 = tmp.tile([128, CH], bf16, tag="t2", name=f"t2q{hp}")
                nc.vector.tensor_mul(t2[:], prot[:], tsin_q[:])
                qc = tmp.tile([128, CH], bf16, tag="qc", name=f"qc{hp}")
                nc.vector.tensor_mul(qc[:], qraw[:], tcos_q[:])
                nc.vector.tensor_add(qT[:], qc[:], t2[:])

                kT = qkp.tile([128, KV], bf16, tag="kT", name=f"kT{hp}")
                for si, (s0, sw) in enumerate(KSEG):
                    prk = psmm.tile([128, 512], f32, tag="mm",
                                    name=f"prk{hp}_{si}")
                    nc.tensor.matmul(prk[:, 0:sw], tp2[:],
                                     kraw[:, s0:s0 + sw], start=True,
                                     stop=True)
                    t2k = tmp.tile([128, 512], bf16, tag="t2",
                                   name=f"t2k{hp}_{si}")
                    nc.vector.tensor_mul(t2k[:, 0:sw], prk[:, 0:sw],
                                         tsin_k[:, s0:s0 + sw])
                    kck = tmp.tile([128, 512], bf16, tag="qc",
                                   name=f"kc{hp}_{si}")
                    nc.vector.tensor_mul(kck[:, 0:sw], kraw[:, s0:s0 + sw],
                                         tcos_k[:, s0:s0 + sw])
                    nc.vector.tensor_add(kT[:, s0:s0 + sw], kck[:, 0:sw],
                                         t2k[:, 0:sw])
                return qT, kT

            qk_state = qkv_rope(0)

            # ---------- V = xT.T @ wv (6 chunks; sink V preloaded) --------
            wvc = []
            for kc in range(8):
                t = big.tile([128, 1024], bf16, tag="big", name=f"wvc{kc}")
                nc.sync.dma_start(t[:], wv[kc * 128:(kc + 1) * 128, :])
                wvc.append(t)
            tvsink = pers.tile([4, 1040], bf16, tag="vsink")
            nc.sync.dma_start(tvsink[:], vsinkd[:])
            tmask = pers.tile([128, MTOT], bf16, tag="mask")
            nc.sync.dma_start(tmask[:], masks[:])

            v_sb = []
            for tt in range(6):
                vt = pers.tile([128, 1040], bf16, tag=f"v{tt}", name=f"v{tt}")
                vr = vt.rearrange("p (h e) -> p h e", e=65)
                pv = [psmm.tile([128, 512], f32, tag="mm", name=f"pv{tt}_{i}")
                      for i in range(2)]
                for kc in range(8):
                    xsl = (xa[kc][:, tt * 128:(tt + 1) * 128] if tt < 4 else
                           xb[kc][:, (tt - 4) * 128:(tt - 3) * 128])
                    for dh in range(2):
                        nc.tensor.matmul(
                            pv[dh][:], xsl,
                            wvc[kc][:, dh * 512:(dh + 1) * 512],
                            start=(kc == 0), stop=(kc == 7),
                        )
                for dh in range(2):
                    nc.scalar.copy(
                        vr[:, dh * 8:(dh + 1) * 8, 0:64],
                        pv[dh][:].rearrange("p (h e) -> p h e", e=64),
                    )
                nc.vector.memset(vr[:, :, 64:65], 1.0)
                v_sb.append(vt)

            # preload projection weights during the loop
            wp_sb = []
            for cc in range(8):
                t = pers.tile([128, 1024], bf16, tag=f"wp{cc}",
                              name=f"wp{cc}")
                nc.sync.dma_start(t[:], wps[cc * 128:(cc + 1) * 128, :])
                wp_sb.append(t)

            # ---------- scores + exp + mask -------------------------------
            # psum pairing: (c2,c3) and (c4,c5) share a tile/activation
            PAIR = {2: (2, 0), 3: (2, 256), 4: (4, 0), 5: (4, 128)}

            def sc_block(hp, qT, kT):
                pts = {}
                for half in range(2):
                    dsl = slice(half * 64, half * 64 + 64)
                    scs = {}
                    for c in SCORD:
                        w, off = W_C[c], OFF_C[c]
                        base, bo = PAIR.get(c, (c, 0))
                        if base not in scs:
                            scs[base] = pssc.tile(
                                [128, 512], f32, tag="sc",
                                name=f"sc{hp}_{base}_{half}")
                        lhsT = (tksink[dsl, hp * 4:hp * 4 + 4] if c == 6 else
                                kT[dsl, c * 128:(c + 1) * 128])
                        rows = 4 if c == 6 else 128
                        nc.tensor.matmul(
                            scs[base][0:rows, bo:bo + w], lhsT,
                            qT[dsl, off:off + w], start=True, stop=True,
                        )
                    for base, tw, rows in ((6, 512, 4), (0, 384, 128),
                                           (1, 384, 128), (2, 384, 128),
                                           (4, 384, 128)):
                        praw = big.tile([128, 512], bf16, tag="big",
                                        name=f"praw{hp}_{base}_{half}")
                        nc.scalar.activation(praw[0:rows, 0:tw],
                                             scs[base][0:rows, 0:tw],
                                             AF.Exp, scale=0.125)
                        pt = ptp.tile([128, 512], bf16, tag="pt",
                                      name=f"pt{hp}_{base}_{half}")
                        eng = nc.vector if half == 0 else nc.gpsimd
                        eng.tensor_mul(
                            pt[0:rows, 0:tw], praw[0:rows, 0:tw],
                            tmask[0:rows, MOFF[base]:MOFF[base] + tw],
                        )
                        pts[(base, half)] = pt
                return pts

            # ---------- AV + per-hp normalization -------------------------
            def av_block(hp, pts):
                yt_pair = []
                for half in range(2):
                    h = hp * 2 + half
                    yt = psyt.tile([65, 512], f32, tag="yt",
                                   name=f"yt{hp}_{half}")
                    for ci, c in enumerate(SCORD):
                        w, off = W_C[c], OFF_C[c]
                        base, bo = PAIR.get(c, (c, 0))
                        lhsT = (tvsink[:, h * 65:(h + 1) * 65] if c == 6 else
                                v_sb[c][:, h * 65:(h + 1) * 65])
                        nc.tensor.matmul(
                            yt[:, off:off + w], lhsT,
                            pts[(base, half)][0:(4 if c == 6 else 128),
                                              bo:bo + w],
                            start=(ci == 0), stop=(ci == 6),
                        )
                    yt_pair.append(yt)
                # denominators -> reciprocal -> PE broadcast -> scale
                rh = []
                for half in range(2):
                    dt1 = smp.tile([1, 512], f32, tag="dt",
                                   name=f"dt{hp}_{half}")
                    nc.scalar.copy(dt1[:], yt_pair[half][64:65, :])
                    r1 = smp.tile([1, 512], f32r, tag="r2",
                                  name=f"r{hp}_{half}")
                    nc.vector.reciprocal(r1[:], dt1[:])
                    rh.append(r1)
                prb = psnb.tile([128, 512], f32, tag="nb", name=f"prb{hp}")
                for half in range(2):
                    nc.tensor.matmul(prb[:], tsel[half][:], rh[half][:],
                                     start=(half == 0), stop=(half == 1))
                ytu = ytsp.tile([128, CH], bf16, tag=f"ytu{hp}",
                                name=f"ytu{hp}")
                nc.scalar.copy(ytu[0:64, :], yt_pair[0][0:64, :])
                nc.scalar.copy(ytu[64:128, :], yt_pair[1][0:64, :])
                nc.vector.tensor_mul(ytu[:], ytu[:], prb[:])
                return ytu

            # ---------- software-pipelined head-pair loop -----------------
            yts = []
            for hp in range(8):
                pts = sc_block(hp, *qk_state)
                if hp < 7:
                    qk_state = qkv_rope(hp + 1)
                yts.append(av_block(hp, pts))

            # ---------- projection (weights preloaded) --------------------
            for cc in range(8):
                po = psmm.tile([128, 512], f32, tag="mm", name=f"po{cc}")
                for hp in range(8):
                    nc.tensor.matmul(
                        po[:], wp_sb[cc][:, hp * 128:(hp + 1) * 128],
                        yts[hp][:],
                        start=(hp == 0), stop=(hp == 7),
                    )
                osb = big.tile([128, 512], f32, tag="osb", name=f"osb{cc}")
                nc.scalar.copy(osb[:], po[:])
                nc.sync.dma_start(outT[cc * 128:(cc + 1) * 128, :], osb[:])

    nc.compile()
    return nc


def _host_inputs(x, w_attn, w_proj):
    """Build the 8 per-core input maps."""
    import ml_dtypes
    bf16 = ml_dtypes.bfloat16

    inv_freq = 1.0 / (10000.0 ** (np.arange(0, HD, 2, dtype=np.float32) / HD))
    iff = np.concatenate([inv_freq, inv_freq])  # [64]

    def cos_sin(pos):
        ang = pos[None, :].astype(np.float32) * iff[:, None]
        c = np.concatenate([np.cos(ang), np.cos(ang)], 0)
        s = np.concatenate([np.sin(ang), np.sin(ang)], 0)
        return (np.ascontiguousarray(c).astype(bf16),
                np.ascontiguousarray(s).astype(bf16))

    def rope_rows(v, pos):
        # v [n, 64] at positions pos -> rope'd [n, 64]
        ang = pos[:, None].astype(np.float32) * iff[None, :]
        cos, sin = np.cos(ang), np.sin(ang)
        rot = np.concatenate([-v[:, 32:], v[:, :32]], axis=1)
        return v * cos + rot * sin

    P2 = np.zeros((128, 128), np.float32)
    for blk in range(2):
        o = blk * 64
        for d in range(32):
            P2[o + d + 32, o + d] = -1.0
            P2[o + d, o + d + 32] = 1.0

    sel2 = np.zeros((2, 128), np.float32)
    sel2[0, 0:64] = 1.0
    sel2[1, 64:128] = 1.0

    def shuffle_lhsT(w):
        return np.ascontiguousarray(
            w.reshape(8, 128, 8, 128).transpose(2, 1, 0, 3).reshape(C, C)
        )

    wq = shuffle_lhsT(w_attn[:, 0:C]).astype(bf16)
    wk = shuffle_lhsT(w_attn[:, C:2 * C]).astype(bf16)
    wvm = np.ascontiguousarray(w_attn[:, 2 * C:3 * C]).astype(bf16)
    wp = shuffle_lhsT(w_proj).astype(bf16)

    # per-batch sink K/V (host-computed, tiny)
    vsink_b, ksink_b = [], []
    for b in range(B):
        k_s = x[b, 0:4] @ w_attn[:, C:2 * C]      # [4, 1024]
        v_s = x[b, 0:4] @ w_attn[:, 2 * C:3 * C]  # [4, 1024]
        vs = np.zeros((4, 1040), np.float32)
        vsr = vs.reshape(4, 16, 65)
        vsr[:, :, 0:64] = v_s.reshape(4, 16, 64)
        vsr[:, :, 64] = 1.0
        vsink_b.append(vs.astype(bf16))
        ks = np.zeros((128, 32), np.float32)
        pos4 = np.arange(4)
        for h in range(NH):
            hp, half = h // 2, h % 2
            kr = rope_rows(k_s[:, h * 64:(h + 1) * 64], pos4)  # [4, 64]
            ks[half * 64:(half + 1) * 64, hp * 4:hp * 4 + 4] = kr.T
        ksink_b.append(ks.astype(bf16))

    in_maps = []
    for core in range(NCORES):
        b, j = core // 4, core % 4
        q0 = j * CH
        # kv columns: [own 512 | halo 256]
        kv_gk = np.full(KV, -1, np.int64)
        kv_gk[0:512] = q0 + np.arange(CH)
        halo = q0 - 256 + np.arange(256)
        kv_gk[512:768] = np.where(halo >= 0, halo, -1)

        xTc = np.zeros((C, KV), np.float32)
        valid = kv_gk >= 0
        xTc[:, valid] = x[b, kv_gk[valid]].T

        cq, sq = cos_sin(q0 + np.arange(CH))
        ck, sk = cos_sin(np.maximum(kv_gk, 0))

        gq = q0 + np.arange(CH)
        mask = np.zeros((128, MTOT), np.float32)
        for c in range(7):
            if c == 6:
                g = np.arange(4)[:, None]                # sink positions
                qq = gq[None, OFF_C[c]:OFF_C[c] + W_C[c]]
                allow = (g <= qq) & (qq - g >= WIN)
                mask[0:4, MOFF[c]:MOFF[c] + W_C[c]] = allow
                continue
            rows = np.arange(128)
            gk = kv_gk[c * 128 + rows]
            qw = gq[OFF_C[c]:OFF_C[c] + W_C[c]]
            real = gk >= 0
            g = np.where(real, gk, 0)[:, None]
            qq = qw[None, :]
            allow = (g <= qq) & (qq - g < WIN) & real[:, None]
            mask[:, MOFF[c]:MOFF[c] + W_C[c]] = allow.astype(np.float32)

        in_maps.append({
            "xT": xTc.astype(bf16), "wqs": wq, "wks": wk, "wv": wvm,
            "wps": wp, "cos_q": cq, "sin_q": sq, "cos_k": ck, "sin_k": sk,
            "masks": mask.astype(bf16), "p2": P2.astype(bf16),
            "vsink": vsink_b[b], "ksink": ksink_b[b], "sel2": sel2,
        })
    return in_maps


def kernel(x, w_attn, w_proj):
    from concourse import bass_utils

    x = np.asarray(x, np.float32)
    w_attn = np.asarray(w_attn, np.float32)
    w_proj = np.asarray(w_proj, np.float32)

    if "nc" not in _cache:
        _cache["nc"] = _build_nc()
    nc = _cache["nc"]

    in_maps = _host_inputs(x, w_attn, w_proj)
    res = bass_utils.run_bass_kernel_spmd(nc, in_maps, list(range(NCORES)),
                                          **_cache.get("run_kwargs", {}))
    _cache["last_result"] = res

    y = np.zeros((B, T, C), np.float32)
    for core in range(NCORES):
        b, j = core // 4, core % 4
        y[b, j * CH:(j + 1) * CH, :] = res.results[core]["outT"].T
    return y


# revision 6
# speedup vs baseline: 1.2367x; 1.0412x over previous
"""Trainium2 Bass kernel for CausalSelfAttention with sliding-window + sink mask.

Sharding: 8 cores = (batch 2) x (sequence chunks of 512). Each core computes
QKV (+RoPE) for its 512 queries and a kv range [512 own | 256 halo] = 768
positions (6 chunks of 128); the 4 attention-sink K/V rows are computed on
the host and uploaded as tiny persistent tiles. Banded attention runs in a
scores-transposed [kv, q] layout with per-chunk q-windows, exp on the scalar
engine, multiplicative 0/1 masking split across vector+gpsimd, denominator
via a ones-column in V, per-head-pair normalization inside the loop (PE
broadcast of a 2-row reciprocal), then a preloaded-weight projection that
emits a transposed [C, 512] output the host re-transposes and concatenates.

All matmul operands are bf16 (full-rate PE path, no sub-256 f32r penalty),
accumulation stays f32 in PSUM.
"""

import numpy as np

B, T, C, NH, HD = 2, 2048, 1024, 16, 64
WIN, SINK = 256, 4
CH = 512          # queries per core
KV = 768          # 512 own + 256 halo (sink handled separately)
NCORES = 8
W_C = [384, 384, 256, 128, 128, 256, 512]
OFF_C = [0, 128, 256, 384, 0, 0, 0]
MOFF = np.concatenate([[0], np.cumsum(W_C)]).astype(int)
MTOT = int(MOFF[-1])  # 2048

_cache = {}


def _build_nc():
    import concourse.bacc as bacc
    import concourse.mybir as mybir
    import concourse.tile as tile

    f32 = mybir.dt.float32
    f32r = mybir.dt.float32r
    bf16 = mybir.dt.bfloat16
    AF = mybir.ActivationFunctionType

    nc = bacc.Bacc("TRN2", target_bir_lowering=False, debug=False,
                   num_devices=NCORES)

    xT = nc.dram_tensor("xT", [C, KV], bf16, kind="ExternalInput").ap()
    wqs = nc.dram_tensor("wqs", [C, C], bf16, kind="ExternalInput").ap()
    wks = nc.dram_tensor("wks", [C, C], bf16, kind="ExternalInput").ap()
    wv = nc.dram_tensor("wv", [C, C], bf16, kind="ExternalInput").ap()
    wps = nc.dram_tensor("wps", [C, C], bf16, kind="ExternalInput").ap()
    cos_q = nc.dram_tensor("cos_q", [128, CH], bf16, kind="ExternalInput").ap()
    sin_q = nc.dram_tensor("sin_q", [128, CH], bf16, kind="ExternalInput").ap()
    cos_k = nc.dram_tensor("cos_k", [128, KV], bf16, kind="ExternalInput").ap()
    sin_k = nc.dram_tensor("sin_k", [128, KV], bf16, kind="ExternalInput").ap()
    masks = nc.dram_tensor("masks", [128, MTOT], bf16,
                           kind="ExternalInput").ap()
    p2d = nc.dram_tensor("p2", [128, 128], bf16, kind="ExternalInput").ap()
    vsinkd = nc.dram_tensor("vsink", [4, 1040], bf16,
                            kind="ExternalInput").ap()
    ksinkd = nc.dram_tensor("ksink", [128, 32], bf16,
                            kind="ExternalInput").ap()
    sel2d = nc.dram_tensor("sel2", [2, 128], f32r, kind="ExternalInput").ap()
    outT = nc.dram_tensor("outT", [C, CH], f32, kind="ExternalOutput").ap()

    KSEG = [(0, 512), (512, 256)]  # kv free-dim segments (psum bank limit)
    SCORD = [6, 0, 1, 2, 3, 4, 5]  # sink first: AV accumulation starts full

    with tile.TileContext(nc) as tc:
        with (
            nc.allow_low_precision(reason="bf16 matmul operands throughout"),
            tc.tile_pool(name="pers", bufs=1) as pers,
            tc.tile_pool(name="wsl", bufs=2) as wsl,
            tc.tile_pool(name="big", bufs=8) as big,     # wv chunks then praw
            tc.tile_pool(name="qk", bufs=2) as qkp,
            tc.tile_pool(name="tmp", bufs=2) as tmp,
            tc.tile_pool(name="yts", bufs=1) as ytsp,
            tc.tile_pool(name="ptp", bufs=10) as ptp,
            tc.tile_pool(name="sm", bufs=4) as smp,
            tc.tile_pool(name="psmm", bufs=2, space="PSUM") as psmm,
            tc.tile_pool(name="pssc", bufs=4, space="PSUM") as pssc,
            tc.tile_pool(name="psyt", bufs=2, space="PSUM") as psyt,
        ):
            # ---------- persistent loads (ordered for fast PE start) ------
            xa, xb = [], []
            for i in range(8):
                t = pers.tile([128, 512], bf16, tag=f"xa{i}", name=f"xa{i}")
                nc.sync.dma_start(t[:], xT[i * 128:(i + 1) * 128, 0:512])
                xa.append(t)
            for i in range(8):
                t = pers.tile([128, 256], bf16, tag=f"xb{i}", name=f"xb{i}")
                nc.sync.dma_start(t[:], xT[i * 128:(i + 1) * 128, 512:768])
                xb.append(t)
            tp2 = pers.tile([128, 128], bf16, tag="p2")
            nc.sync.dma_start(tp2[:], p2d[:])
            tcos_q = pers.tile([128, CH], bf16, tag="cos_q")
            nc.sync.dma_start(tcos_q[:], cos_q[:])
            tsin_q = pers.tile([128, CH], bf16, tag="sin_q")
            nc.sync.dma_start(tsin_q[:], sin_q[:])
            tcos_k = pers.tile([128, KV], bf16, tag="cos_k")
            nc.sync.dma_start(tcos_k[:], cos_k[:])
            tsin_k = pers.tile([128, KV], bf16, tag="sin_k")
            nc.sync.dma_start(tsin_k[:], sin_k[:])
            tksink = pers.tile([128, 32], bf16, tag="ksink")
            nc.sync.dma_start(tksink[:], ksinkd[:])
            tsel = []
            for half in range(2):
                t = pers.tile([1, 128], f32r, tag=f"sel{half}")
                nc.sync.dma_start(t[:], sel2d[half:half + 1, :])
                tsel.append(t)

            # ---------- qkv + rope (emitted per head-pair) ----------------
            def qkv_rope(hp):
                wq_sl = wsl.tile([128, 1024], bf16, tag="wslab",
                                 name=f"wq{hp}")
                nc.sync.dma_start(wq_sl[:], wqs[hp * 128:(hp + 1) * 128, :])
                pq = psmm.tile([128, 512], f32, tag="mm", name=f"pq{hp}")
                for kc in range(8):
                    nc.tensor.matmul(
                        pq[:], wq_sl[:, kc * 128:(kc + 1) * 128],
                        xa[kc][:],
                        start=(kc == 0), stop=(kc == 7),
                    )
                qraw = tmp.tile([128, CH], bf16, tag="qraw", name=f"qraw{hp}")
                nc.scalar.copy(qraw[:], pq[:])

                wk_sl = wsl.tile([128, 1024], bf16, tag="wslab",
                                 name=f"wk{hp}")
                nc.sync.dma_start(wk_sl[:], wks[hp * 128:(hp + 1) * 128, :])
                kraw = tmp.tile([128, KV], bf16, tag="kraw", name=f"kraw{hp}")
                pk = [psmm.tile([128, 512], f32, tag="mm", name=f"pk{hp}_{i}")
                      for i in range(2)]
                for kc in range(8):
                    for si, (s0, sw) in enumerate(KSEG):
                        rhs = xa[kc][:] if si == 0 else xb[kc][:]
                        nc.tensor.matmul(
                            pk[si][:, 0:sw], wk_sl[:, kc * 128:(kc + 1) * 128],
                            rhs, start=(kc == 0), stop=(kc == 7),
                        )
                for si, (s0, sw) in enumerate(KSEG):
                    nc.scalar.copy(kraw[:, s0:s0 + sw], pk[si][:, 0:sw])

                # rope: out = raw*cos + (P2@raw)*sin ; all-bf16 muls get DVE 2x
                qT = qkp.tile([128, CH], bf16, tag="qT", name=f"qT{hp}")
                prot = psmm.tile([128, 512], f32, tag="mm", name=f"prot{hp}")
                nc.tensor.matmul(prot[:], tp2[:], qraw[:], start=True,
                                 stop=True)
                t2 = tmp.tile([128, CH], bf16, tag="t2", name=f"t2q{hp}")
                nc.vector.tensor_mul(t2[:], prot[:], tsin_q[:])
                qc = tmp.tile([128, CH], bf16, tag="qc", name=f"qc{hp}")
                nc.vector.tensor_mul(qc[:], qraw[:], tcos_q[:])
                nc.vector.tensor_add(qT[:], qc[:], t2[:])

                kT = qkp.tile([128, KV], bf16, tag="kT", name=f"kT{hp}")
                for si, (s0, sw) in enumerate(KSEG):
                    prk = psmm.tile([128, 512], f32, tag="mm",
                                    name=f"prk{hp}_{si}")
                    nc.tensor.matmul(prk[:, 0:sw], tp2[:],
                                     kraw[:, s0:s0 + sw], start=True,
                                     stop=True)
                    t2k = tmp.tile([128, 512], bf16, tag="t2",
                                   name=f"t2k{hp}_{si}")
                    nc.vector.tensor_mul(t2k[:, 0:sw], prk[:, 0:sw],
                                         tsin_k[:, s0:s0 + sw])
                    kck = tmp.tile([128, 512], bf16, tag="qc",
                                   name=f"kc{hp}_{si}")
                    nc.vector.tensor_mul(kck[:, 0:sw], kraw[:, s0:s0 + sw],
                                         tcos_k[:, s0:s0 + sw])
                    nc.vector.tensor_add(kT[:, s0:s0 + sw], kck[:, 0:sw],
                                         t2k[:, 0:sw])
                return qT, kT

            qk_state = qkv_rope(0)

            # ---------- V = xT.T @ wv (6 chunks; sink V preloaded) --------
            wvc = []
            for kc in range(8):
                t = big.tile([128, 1024], bf16, tag="big", name=f"wvc{kc}")
                nc.sync.dma_start(t[:], wv[kc * 128:(kc + 1) * 128, :])
                wvc.append(t)
            tvsink = pers.tile([4, 1040], bf16, tag="vsink")
            nc.sync.dma_start(tvsink[:], vsinkd[:])
            tmask = pers.tile([128, MTOT], bf16, tag="mask")
            nc.sync.dma_start(tmask[:], masks[:])

            v_sb = []
            for tt in range(6):
                vt = pers.tile([128, 1040], bf16, tag=f"v{tt}", name=f"v{tt}")
                vr = vt.rearrange("p (h e) -> p h e", e=65)
                pv = [psmm.tile([128, 512], f32, tag="mm", name=f"pv{tt}_{i}")
                      for i in range(2)]
                for kc in range(8):
                    xsl = (xa[kc][:, tt * 128:(tt + 1) * 128] if tt < 4 else
                           xb[kc][:, (tt - 4) * 128:(tt - 3) * 128])
                    for dh in range(2):
                        nc.tensor.matmul(
                            pv[dh][:], xsl,
                            wvc[kc][:, dh * 512:(dh + 1) * 512],
                            start=(kc == 0), stop=(kc == 7),
                        )
                for dh in range(2):
                    nc.scalar.copy(
                        vr[:, dh * 8:(dh + 1) * 8, 0:64],
                        pv[dh][:].rearrange("p (h e) -> p h e", e=64),
                    )
                nc.vector.memset(vr[:, :, 64:65], 1.0)
                v_sb.append(vt)

            # preload projection weights during the loop
            wp_sb = []
            for cc in range(8):
                t = pers.tile([128, 1024], bf16, tag=f"wp{cc}",
                              name=f"wp{cc}")
                nc.sync.dma_start(t[:], wps[cc * 128:(cc + 1) * 128, :])
                wp_sb.append(t)

            # ---------- scores + exp + mask -------------------------------
            # psum pairing: (c2,c3) and (c4,c5) share a tile/activation
            PAIR = {2: (2, 0), 3: (2, 256), 4: (4, 0), 5: (4, 128)}

            def sc_block(hp, qT, kT):
                pts = {}
                for half in range(2):
                    dsl = slice(half * 64, half * 64 + 64)
                    scs = {}
                    for c in SCORD:
                        w, off = W_C[c], OFF_C[c]
                        base, bo = PAIR.get(c, (c, 0))
                        if base not in scs:
                            scs[base] = pssc.tile(
                                [128, 512], f32, tag="sc",
                                name=f"sc{hp}_{base}_{half}")
                        lhsT = (tksink[dsl, hp * 4:hp * 4 + 4] if c == 6 else
                                kT[dsl, c * 128:(c + 1) * 128])
                        rows = 4 if c == 6 else 128
                        nc.tensor.matmul(
                            scs[base][0:rows, bo:bo + w], lhsT,
                            qT[dsl, off:off + w], start=True, stop=True,
                        )
                    for base, tw, rows in ((6, 512, 4), (0, 384, 128),
                                           (1, 384, 128), (2, 384, 128),
                                           (4, 384, 128)):
                        praw = big.tile([128, 512], bf16, tag="big",
                                        name=f"praw{hp}_{base}_{half}")
                        nc.scalar.activation(praw[0:rows, 0:tw],
                                             scs[base][0:rows, 0:tw],
                                             AF.Exp, scale=0.125)
                        pt = ptp.tile([128, 512], bf16, tag="pt",
                                      name=f"pt{hp}_{base}_{half}")
                        eng = nc.vector if half == 0 else nc.gpsimd
                        eng.tensor_mul(
                            pt[0:rows, 0:tw], praw[0:rows, 0:tw],
                            tmask[0:rows, MOFF[base]:MOFF[base] + tw],
                        )
                        pts[(base, half)] = pt
                return pts

            # ---------- AV + per-hp normalization -------------------------
            def av_block(hp, pts):
                yt_pair = []
                for half in range(2):
                    h = hp * 2 + half
                    yt = psyt.tile([65, 512], f32, tag="yt",
                                   name=f"yt{hp}_{half}")
                    for ci, c in enumerate(SCORD):
                        w, off = W_C[c], OFF_C[c]
                        base, bo = PAIR.get(c, (c, 0))
                        lhsT = (tvsink[:, h * 65:(h + 1) * 65] if c == 6 else
                                v_sb[c][:, h * 65:(h + 1) * 65])
                        nc.tensor.matmul(
                            yt[:, off:off + w], lhsT,
                            pts[(base, half)][0:(4 if c == 6 else 128),
                                              bo:bo + w],
                            start=(ci == 0), stop=(ci == 6),
                        )
                    yt_pair.append(yt)
                # denominators -> reciprocal issued now; prb + scale deferred
                # one hp so the in-order PE queue never waits on reciprocal
                rh = []
                for half in range(2):
                    dt1 = smp.tile([1, 512], f32, tag="dt",
                                   name=f"dt{hp}_{half}")
                    nc.scalar.copy(dt1[:], yt_pair[half][64:65, :])
                    r1 = smp.tile([1, 512], f32r, tag="r2",
                                  name=f"r{hp}_{half}")
                    nc.vector.reciprocal(r1[:], dt1[:])
                    rh.append(r1)
                ytu = ytsp.tile([128, CH], bf16, tag=f"ytu{hp}",
                                name=f"ytu{hp}")
                nc.scalar.copy(ytu[0:64, :], yt_pair[0][0:64, :])
                nc.scalar.copy(ytu[64:128, :], yt_pair[1][0:64, :])
                return ytu, rh

            def apply_norm(hp, ytu, rh):
                prb = psmm.tile([128, 512], f32, tag="mm", name=f"prb{hp}")
                for half in range(2):
                    nc.tensor.matmul(prb[:], tsel[half][:], rh[half][:],
                                     start=(half == 0), stop=(half == 1))
                nc.vector.tensor_mul(ytu[:], ytu[:], prb[:])
                return ytu

            # ---------- software-pipelined head-pair loop -----------------
            yts = []
            pend = None
            for hp in range(8):
                pts = sc_block(hp, *qk_state)
                if hp < 7:
                    qk_state = qkv_rope(hp + 1)
                ytu, rh = av_block(hp, pts)
                if pend is not None:
                    yts.append(apply_norm(*pend))
                pend = (hp, ytu, rh)
            yts.append(apply_norm(*pend))

            # ---------- projection (weights preloaded) --------------------
            for cc in range(8):
                po = psmm.tile([128, 512], f32, tag="mm", name=f"po{cc}")
                for hp in range(8):
                    nc.tensor.matmul(
                        po[:], wp_sb[cc][:, hp * 128:(hp + 1) * 128],
                        yts[hp][:],
                        start=(hp == 0), stop=(hp == 7),
                    )
                osb = big.tile([128, 512], f32, tag="osb", name=f"osb{cc}")
                nc.scalar.copy(osb[:], po[:])
                nc.sync.dma_start(outT[cc * 128:(cc + 1) * 128, :], osb[:])

    nc.compile()
    return nc


def _host_inputs(x, w_attn, w_proj):
    """Build the 8 per-core input maps."""
    import ml_dtypes
    bf16 = ml_dtypes.bfloat16

    inv_freq = 1.0 / (10000.0 ** (np.arange(0, HD, 2, dtype=np.float32) / HD))
    iff = np.concatenate([inv_freq, inv_freq])  # [64]

    def cos_sin(pos):
        ang = pos[None, :].astype(np.float32) * iff[:, None]
        c = np.concatenate([np.cos(ang), np.cos(ang)], 0)
        s = np.concatenate([np.sin(ang), np.sin(ang)], 0)
        return (np.ascontiguousarray(c).astype(bf16),
                np.ascontiguousarray(s).astype(bf16))

    def rope_rows(v, pos):
        # v [n, 64] at positions pos -> rope'd [n, 64]
        ang = pos[:, None].astype(np.float32) * iff[None, :]
        cos, sin = np.cos(ang), np.sin(ang)
        rot = np.concatenate([-v[:, 32:], v[:, :32]], axis=1)
        return v * cos + rot * sin

    P2 = np.zeros((128, 128), np.float32)
    for blk in range(2):
        o = blk * 64
        for d in range(32):
            P2[o + d + 32, o + d] = -1.0
            P2[o + d, o + d + 32] = 1.0

    sel2 = np.zeros((2, 128), np.float32)
    sel2[0, 0:64] = 1.0
    sel2[1, 64:128] = 1.0

    def shuffle_lhsT(w):
        return np.ascontiguousarray(
            w.reshape(8, 128, 8, 128).transpose(2, 1, 0, 3).reshape(C, C)
        )

    wq = shuffle_lhsT(w_attn[:, 0:C]).astype(bf16)
    wk = shuffle_lhsT(w_attn[:, C:2 * C]).astype(bf16)
    wvm = np.ascontiguousarray(w_attn[:, 2 * C:3 * C]).astype(bf16)
    wp = shuffle_lhsT(w_proj).astype(bf16)

    # per-batch sink K/V (host-computed, tiny)
    vsink_b, ksink_b = [], []
    for b in range(B):
        k_s = x[b, 0:4] @ w_attn[:, C:2 * C]      # [4, 1024]
        v_s = x[b, 0:4] @ w_attn[:, 2 * C:3 * C]  # [4, 1024]
        vs = np.zeros((4, 1040), np.float32)
        vsr = vs.reshape(4, 16, 65)
        vsr[:, :, 0:64] = v_s.reshape(4, 16, 64)
        vsr[:, :, 64] = 1.0
        vsink_b.append(vs.astype(bf16))
        ks = np.zeros((128, 32), np.float32)
        pos4 = np.arange(4)
        for h in range(NH):
            hp, half = h // 2, h % 2
            kr = rope_rows(k_s[:, h * 64:(h + 1) * 64], pos4)  # [4, 64]
            ks[half * 64:(half + 1) * 64, hp * 4:hp * 4 + 4] = kr.T
        ksink_b.append(ks.astype(bf16))

    in_maps = []
    for core in range(NCORES):
        b, j = core // 4, core % 4
        q0 = j * CH
        # kv columns: [own 512 | halo 256]
        kv_gk = np.full(KV, -1, np.int64)
        kv_gk[0:512] = q0 + np.arange(CH)
        halo = q0 - 256 + np.arange(256)
        kv_gk[512:768] = np.where(halo >= 0, halo, -1)

        xTc = np.zeros((C, KV), np.float32)
        valid = kv_gk >= 0
        xTc[:, valid] = x[b, kv_gk[valid]].T

        cq, sq = cos_sin(q0 + np.arange(CH))
        ck, sk = cos_sin(np.maximum(kv_gk, 0))

        gq = q0 + np.arange(CH)
        mask = np.zeros((128, MTOT), np.float32)
        for c in range(7):
            if c == 6:
                g = np.arange(4)[:, None]                # sink positions
                qq = gq[None, OFF_C[c]:OFF_C[c] + W_C[c]]
                allow = (g <= qq) & (qq - g >= WIN)
                mask[0:4, MOFF[c]:MOFF[c] + W_C[c]] = allow
                continue
            rows = np.arange(128)
            gk = kv_gk[c * 128 + rows]
            qw = gq[OFF_C[c]:OFF_C[c] + W_C[c]]
            real = gk >= 0
            g = np.where(real, gk, 0)[:, None]
            qq = qw[None, :]
            allow = (g <= qq) & (qq - g < WIN) & real[:, None]
            mask[:, MOFF[c]:MOFF[c] + W_C[c]] = allow.astype(np.float32)

        in_maps.append({
            "xT": xTc.astype(bf16), "wqs": wq, "wks": wk, "wv": wvm,
            "wps": wp, "cos_q": cq, "sin_q": sq, "cos_k": ck, "sin_k": sk,
            "masks": mask.astype(bf16), "p2": P2.astype(bf16),
            "vsink": vsink_b[b], "ksink": ksink_b[b], "sel2": sel2,
        })
    return in_maps


def kernel(x, w_attn, w_proj):
    from concourse import bass_utils

    x = np.asarray(x, np.float32)
    w_attn = np.asarray(w_attn, np.float32)
    w_proj = np.asarray(w_proj, np.float32)

    if "nc" not in _cache:
        _cache["nc"] = _build_nc()
    nc = _cache["nc"]

    in_maps = _host_inputs(x, w_attn, w_proj)
    res = bass_utils.run_bass_kernel_spmd(nc, in_maps, list(range(NCORES)),
                                          **_cache.get("run_kwargs", {}))
    _cache["last_result"] = res

    y = np.zeros((B, T, C), np.float32)
    for core in range(NCORES):
        b, j = core // 4, core % 4
        y[b, j * CH:(j + 1) * CH, :] = res.results[core]["outT"].T
    return y


# revision 20
# speedup vs baseline: 1.3046x; 1.0549x over previous
"""Trainium2 Bass kernel for CausalSelfAttention with sliding-window + sink mask.

Sharding: 8 cores = (batch 2) x (sequence chunks of 512). Each core computes
QKV (+RoPE) for its 512 queries and a kv range [512 own | 256 halo] = 768
positions (6 chunks of 128); the 4 attention-sink K/V rows are computed on
the host and uploaded as tiny persistent tiles. Banded attention runs in a
scores-transposed [kv, q] layout with per-chunk q-windows, exp on the scalar
engine, multiplicative 0/1 masking split across vector+gpsimd, denominator
via a ones-column in V, per-head-pair normalization inside the loop (PE
broadcast of a 2-row reciprocal), then a preloaded-weight projection that
emits a transposed [C, 512] output the host re-transposes and concatenates.

All matmul operands are bf16 (full-rate PE path, no sub-256 f32r penalty),
accumulation stays f32 in PSUM.
"""

import numpy as np

B, T, C, NH, HD = 2, 2048, 1024, 16, 64
WIN, SINK = 256, 4
CH = 512          # queries per core
KV = 768          # 512 own + 256 halo (sink handled separately)
NCORES = 8
W_C = [384, 384, 256, 128, 128, 256, 512]
OFF_C = [0, 128, 256, 384, 0, 0, 0]
MOFF = np.concatenate([[0], np.cumsum(W_C)]).astype(int)
MTOT = int(MOFF[-1])  # 2048

_cache = {}


def _build_nc():
    import concourse.bacc as bacc
    import concourse.mybir as mybir
    import concourse.tile as tile

    f32 = mybir.dt.float32
    f32r = mybir.dt.float32r
    bf16 = mybir.dt.bfloat16
    AF = mybir.ActivationFunctionType

    nc = bacc.Bacc("TRN2", target_bir_lowering=False, debug=False,
                   num_devices=NCORES)

    # packed layouts (host pre-shuffles) so startup is a handful of DMAs
    xad = nc.dram_tensor("xad", [128, 4096], bf16, kind="ExternalInput").ap()
    xbd = nc.dram_tensor("xbd", [128, 2048], bf16, kind="ExternalInput").ap()
    wqs = nc.dram_tensor("wqs", [C, C], bf16, kind="ExternalInput").ap()
    wks = nc.dram_tensor("wks", [C, C], bf16, kind="ExternalInput").ap()
    wvd = nc.dram_tensor("wvd", [128, 8192], bf16, kind="ExternalInput").ap()
    wps = nc.dram_tensor("wps", [C, C], bf16, kind="ExternalInput").ap()
    # trig blob: p2 | cos_q | sin_q | cos_k | sin_k | ksink
    trig = nc.dram_tensor("trig", [128, 2720], bf16,
                          kind="ExternalInput").ap()
    masks = nc.dram_tensor("masks", [128, MTOT], bf16,
                           kind="ExternalInput").ap()
    vsinkd = nc.dram_tensor("vsink", [4, 1040], bf16,
                            kind="ExternalInput").ap()
    sel2d = nc.dram_tensor("sel2", [2, 128], f32r, kind="ExternalInput").ap()
    outT = nc.dram_tensor("outT", [C, CH], f32, kind="ExternalOutput").ap()

    KSEG = [(0, 512), (512, 256)]  # kv free-dim segments (psum bank limit)
    SCORD = [6, 0, 1, 2, 3, 4, 5]  # sink first: AV accumulation starts full

    with tile.TileContext(nc) as tc:
        with (
            nc.allow_low_precision(reason="bf16 matmul operands throughout"),
            tc.tile_pool(name="pers", bufs=1) as pers,
            tc.tile_pool(name="wsl", bufs=2) as wsl,
            tc.tile_pool(name="big", bufs=8) as big,     # praw/osb rings
            tc.tile_pool(name="wvp", bufs=1) as wvpool,  # packed wv (1 tile)
            tc.tile_pool(name="qk", bufs=2) as qkp,
            tc.tile_pool(name="tmp", bufs=2) as tmp,
            tc.tile_pool(name="yts", bufs=1) as ytsp,
            tc.tile_pool(name="ptp", bufs=10) as ptp,
            tc.tile_pool(name="sm", bufs=4) as smp,
            tc.tile_pool(name="psmm", bufs=2, space="PSUM") as psmm,
            tc.tile_pool(name="pssc", bufs=4, space="PSUM") as pssc,
            tc.tile_pool(name="psyt", bufs=2, space="PSUM") as psyt,
        ):
            # ---------- persistent loads (ordered for fast PE start) ------
            xa_all = pers.tile([128, 4096], bf16, tag="xa")
            nc.sync.dma_start(xa_all[:], xad[:])
            # wq0 DMA is emitted just below; Q0 needs only xa_all+wq0
            xb_all = pers.tile([128, 2048], bf16, tag="xb")
            ttrig = pers.tile([128, 2720], bf16, tag="trig")
            tp2 = ttrig[:, 0:128]
            tcos_q = ttrig[:, 128:640]
            tsin_q = ttrig[:, 640:1152]
            tcos_k = ttrig[:, 1152:1920]
            tsin_k = ttrig[:, 1920:2688]
            tksink = ttrig[:, 2688:2720]
            tsel = []

            def late_loads():
                nc.sync.dma_start(xb_all[:], xbd[:])
                nc.sync.dma_start(ttrig[:], trig[:])
                for half in range(2):
                    t = pers.tile([1, 128], f32r, tag=f"sel{half}")
                    nc.sync.dma_start(t[:], sel2d[half:half + 1, :])
                    tsel.append(t)

            # ---------- qkv + rope (emitted per head-pair) ----------------
            def qkv_rope(hp, wq_pre=None):
                if wq_pre is None:
                    wq_sl = wsl.tile([128, 1024], bf16, tag="wslab",
                                     name=f"wq{hp}")
                    nc.sync.dma_start(wq_sl[:],
                                      wqs[hp * 128:(hp + 1) * 128, :])
                else:
                    wq_sl = wq_pre
                pq = psmm.tile([128, 512], f32, tag="mm", name=f"pq{hp}")
                for kc in range(8):
                    nc.tensor.matmul(
                        pq[:], wq_sl[:, kc * 128:(kc + 1) * 128],
                        xa_all[:, kc * 512:(kc + 1) * 512],
                        start=(kc == 0), stop=(kc == 7),
                    )
                qraw = tmp.tile([128, CH], bf16, tag="qraw", name=f"qraw{hp}")
                nc.scalar.copy(qraw[:], pq[:])

                wk_sl = wsl.tile([128, 1024], bf16, tag="wslab",
                                 name=f"wk{hp}")
                nc.sync.dma_start(wk_sl[:], wks[hp * 128:(hp + 1) * 128, :])
                kraw = tmp.tile([128, KV], bf16, tag="kraw", name=f"kraw{hp}")
                pk = [psmm.tile([128, 512], f32, tag="mm", name=f"pk{hp}_{i}")
                      for i in range(2)]
                for kc in range(8):
                    for si, (s0, sw) in enumerate(KSEG):
                        rhs = (xa_all[:, kc * 512:(kc + 1) * 512] if si == 0
                               else xb_all[:, kc * 256:(kc + 1) * 256])
                        nc.tensor.matmul(
                            pk[si][:, 0:sw], wk_sl[:, kc * 128:(kc + 1) * 128],
                            rhs, start=(kc == 0), stop=(kc == 7),
                        )
                for si, (s0, sw) in enumerate(KSEG):
                    nc.scalar.copy(kraw[:, s0:s0 + sw], pk[si][:, 0:sw])

                # rope: out = raw*cos + (P2@raw)*sin ; all-bf16 muls get DVE 2x
                qT = qkp.tile([128, CH], bf16, tag="qT", name=f"qT{hp}")
                prot = psmm.tile([128, 512], f32, tag="mm", name=f"prot{hp}")
                nc.tensor.matmul(prot[:], tp2[:], qraw[:], start=True,
                                 stop=True)
                t2 = tmp.tile([128, CH], bf16, tag="t2", name=f"t2q{hp}")
                nc.vector.tensor_mul(t2[:], prot[:], tsin_q[:])
                qc = tmp.tile([128, CH], bf16, tag="qc", name=f"qc{hp}")
                nc.vector.tensor_mul(qc[:], qraw[:], tcos_q[:])
                nc.vector.tensor_add(qT[:], qc[:], t2[:])

                kT = qkp.tile([128, KV], bf16, tag="kT", name=f"kT{hp}")
                for si, (s0, sw) in enumerate(KSEG):
                    prk = psmm.tile([128, 512], f32, tag="mm",
                                    name=f"prk{hp}_{si}")
                    nc.tensor.matmul(prk[:, 0:sw], tp2[:],
                                     kraw[:, s0:s0 + sw], start=True,
                                     stop=True)
                    t2k = tmp.tile([128, 512], bf16, tag="t2",
                                   name=f"t2k{hp}_{si}")
                    nc.vector.tensor_mul(t2k[:, 0:sw], prk[:, 0:sw],
                                         tsin_k[:, s0:s0 + sw])
                    kck = tmp.tile([128, 512], bf16, tag="qc",
                                   name=f"kc{hp}_{si}")
                    nc.vector.tensor_mul(kck[:, 0:sw], kraw[:, s0:s0 + sw],
                                         tcos_k[:, s0:s0 + sw])
                    nc.vector.tensor_add(kT[:, s0:s0 + sw], kck[:, 0:sw],
                                         t2k[:, 0:sw])
                return qT, kT

            wq0 = wsl.tile([128, 1024], bf16, tag="wslab", name="wq0")
            nc.sync.dma_start(wq0[:], wqs[0:128, :])
            late_loads()
            qk_state = qkv_rope(0, wq_pre=wq0)

            # ---------- V = xT.T @ wv (6 chunks; sink V preloaded) --------
            wv_all = wvpool.tile([128, 8192], bf16, tag="wvall",
                                 name="wvall")
            nc.sync.dma_start(wv_all[:], wvd[:])
            wvc = [wv_all[:, kc * 1024:(kc + 1) * 1024] for kc in range(8)]
            tvsink = pers.tile([4, 1040], bf16, tag="vsink")
            nc.sync.dma_start(tvsink[:], vsinkd[:])
            tmask = pers.tile([128, MTOT], bf16, tag="mask")
            nc.sync.dma_start(tmask[:], masks[:])

            v_sb = []
            for tt in range(6):
                vt = pers.tile([128, 1040], bf16, tag=f"v{tt}", name=f"v{tt}")
                vr = vt.rearrange("p (h e) -> p h e", e=65)
                pv = [psmm.tile([128, 512], f32, tag="mm", name=f"pv{tt}_{i}")
                      for i in range(2)]
                for kc in range(8):
                    xsl = (xa_all[:, kc * 512 + tt * 128:
                                  kc * 512 + (tt + 1) * 128] if tt < 4 else
                           xb_all[:, kc * 256 + (tt - 4) * 128:
                                  kc * 256 + (tt - 3) * 128])
                    for dh in range(2):
                        nc.tensor.matmul(
                            pv[dh][:], xsl,
                            wvc[kc][:, dh * 512:(dh + 1) * 512],
                            start=(kc == 0), stop=(kc == 7),
                        )
                for dh in range(2):
                    nc.scalar.copy(
                        vr[:, dh * 8:(dh + 1) * 8, 0:64],
                        pv[dh][:].rearrange("p (h e) -> p h e", e=64),
                    )
                nc.vector.memset(vr[:, :, 64:65], 1.0)
                v_sb.append(vt)

            # preload projection weights during the loop
            wp_sb = []
            for cc in range(8):
                t = pers.tile([128, 1024], bf16, tag=f"wp{cc}",
                              name=f"wp{cc}")
                nc.sync.dma_start(t[:], wps[cc * 128:(cc + 1) * 128, :])
                wp_sb.append(t)

            # ---------- scores + exp + mask -------------------------------
            # psum pairing: (c2,c3) and (c4,c5) share a tile/activation
            PAIR = {2: (2, 0), 3: (2, 256), 4: (4, 0), 5: (4, 128)}

            def sc_block(hp, qT, kT):
                pts = {}
                for half in range(2):
                    dsl = slice(half * 64, half * 64 + 64)
                    scs = {}
                    for c in SCORD:
                        w, off = W_C[c], OFF_C[c]
                        base, bo = PAIR.get(c, (c, 0))
                        if base not in scs:
                            scs[base] = pssc.tile(
                                [128, 512], f32, tag="sc",
                                name=f"sc{hp}_{base}_{half}")
                        lhsT = (tksink[dsl, hp * 4:hp * 4 + 4] if c == 6 else
                                kT[dsl, c * 128:(c + 1) * 128])
                        rows = 4 if c == 6 else 128
                        nc.tensor.matmul(
                            scs[base][0:rows, bo:bo + w], lhsT,
                            qT[dsl, off:off + w], start=True, stop=True,
                        )
                    for base, tw, rows in ((6, 512, 4), (0, 384, 128),
                                           (1, 384, 128), (2, 384, 128),
                                           (4, 384, 128)):
                        praw = big.tile([128, 512], bf16, tag="big",
                                        name=f"praw{hp}_{base}_{half}")
                        nc.scalar.activation(praw[0:rows, 0:tw],
                                             scs[base][0:rows, 0:tw],
                                             AF.Exp, scale=0.125)
                        pt = ptp.tile([128, 512], bf16, tag="pt",
                                      name=f"pt{hp}_{base}_{half}")
                        # gpsimd takes a minority share (it runs ~2ns/elem);
                        # DVE bf16-2x handles the rest without stalling AV
                        eng = (nc.gpsimd if half == 1 and base in (0, 1)
                               else nc.vector)
                        eng.tensor_mul(
                            pt[0:rows, 0:tw], praw[0:rows, 0:tw],
                            tmask[0:rows, MOFF[base]:MOFF[base] + tw],
                        )
                        pts[(base, half)] = pt
                return pts

            # ---------- AV + per-hp normalization -------------------------
            def av_block(hp, pts):
                yt_pair = []
                for half in range(2):
                    h = hp * 2 + half
                    yt = psyt.tile([65, 512], f32, tag="yt",
                                   name=f"yt{hp}_{half}")
                    for ci, c in enumerate(SCORD):
                        w, off = W_C[c], OFF_C[c]
                        base, bo = PAIR.get(c, (c, 0))
                        lhsT = (tvsink[:, h * 65:(h + 1) * 65] if c == 6 else
                                v_sb[c][:, h * 65:(h + 1) * 65])
                        nc.tensor.matmul(
                            yt[:, off:off + w], lhsT,
                            pts[(base, half)][0:(4 if c == 6 else 128),
                                              bo:bo + w],
                            start=(ci == 0), stop=(ci == 6),
                        )
                    yt_pair.append(yt)
                # denominators -> reciprocal issued now; prb + scale deferred
                # one hp so the in-order PE queue never waits on reciprocal
                rh = []
                for half in range(2):
                    dt1 = smp.tile([1, 512], f32, tag="dt",
                                   name=f"dt{hp}_{half}")
                    nc.scalar.copy(dt1[:], yt_pair[half][64:65, :])
                    r1 = smp.tile([1, 512], f32r, tag="r2",
                                  name=f"r{hp}_{half}")
                    nc.vector.reciprocal(r1[:], dt1[:])
                    rh.append(r1)
                ytu = ytsp.tile([128, CH], bf16, tag=f"ytu{hp}",
                                name=f"ytu{hp}")
                nc.scalar.copy(ytu[0:64, :], yt_pair[0][0:64, :])
                nc.scalar.copy(ytu[64:128, :], yt_pair[1][0:64, :])
                return ytu, rh

            def apply_norm(hp, ytu, rh):
                prb = psmm.tile([128, 512], f32, tag="mm", name=f"prb{hp}")
                for half in range(2):
                    nc.tensor.matmul(prb[:], tsel[half][:], rh[half][:],
                                     start=(half == 0), stop=(half == 1))
                nc.vector.tensor_mul(ytu[:], ytu[:], prb[:])
                return ytu

            # ---------- software-pipelined head-pair loop -----------------
            yts = []
            pend = None
            for hp in range(8):
                pts = sc_block(hp, *qk_state)
                if hp < 7:
                    qk_state = qkv_rope(hp + 1)
                ytu, rh = av_block(hp, pts)
                if pend is not None:
                    yts.append(apply_norm(*pend))
                pend = (hp, ytu, rh)
            yts.append(apply_norm(*pend))

            # ---------- projection (weights preloaded) --------------------
            for cc in range(8):
                po = psmm.tile([128, 512], f32, tag="mm", name=f"po{cc}")
                for hp in range(8):
                    nc.tensor.matmul(
                        po[:], wp_sb[cc][:, hp * 128:(hp + 1) * 128],
                        yts[hp][:],
                        start=(hp == 0), stop=(hp == 7),
                    )
                osb = big.tile([128, 512], f32, tag="osb", name=f"osb{cc}")
                nc.scalar.copy(osb[:], po[:])
                nc.sync.dma_start(outT[cc * 128:(cc + 1) * 128, :], osb[:])

    nc.compile()
    return nc


def _host_inputs(x, w_attn, w_proj):
    """Build the 8 per-core input maps."""
    import ml_dtypes
    bf16 = ml_dtypes.bfloat16

    inv_freq = 1.0 / (10000.0 ** (np.arange(0, HD, 2, dtype=np.float32) / HD))
    iff = np.concatenate([inv_freq, inv_freq])  # [64]

    def cos_sin(pos):
        ang = pos[None, :].astype(np.float32) * iff[:, None]
        c = np.concatenate([np.cos(ang), np.cos(ang)], 0)
        s = np.concatenate([np.sin(ang), np.sin(ang)], 0)
        return (np.ascontiguousarray(c).astype(bf16),
                np.ascontiguousarray(s).astype(bf16))

    def rope_rows(v, pos):
        # v [n, 64] at positions pos -> rope'd [n, 64]
        ang = pos[:, None].astype(np.float32) * iff[None, :]
        cos, sin = np.cos(ang), np.sin(ang)
        rot = np.concatenate([-v[:, 32:], v[:, :32]], axis=1)
        return v * cos + rot * sin

    P2 = np.zeros((128, 128), np.float32)
    for blk in range(2):
        o = blk * 64
        for d in range(32):
            P2[o + d + 32, o + d] = -1.0
            P2[o + d, o + d + 32] = 1.0

    sel2 = np.zeros((2, 128), np.float32)
    sel2[0, 0:64] = 1.0
    sel2[1, 64:128] = 1.0

    def shuffle_lhsT(w):
        return np.ascontiguousarray(
            w.reshape(8, 128, 8, 128).transpose(2, 1, 0, 3).reshape(C, C)
        )

    def pack_rows(w, colw):
        # [1024, colw] -> [128, 8*colw]: out[p, kc*colw+t] = w[kc*128+p, t]
        return np.ascontiguousarray(
            w.reshape(8, 128, colw).transpose(1, 0, 2).reshape(128, 8 * colw)
        )

    wq = shuffle_lhsT(w_attn[:, 0:C]).astype(bf16)
    wk = shuffle_lhsT(w_attn[:, C:2 * C]).astype(bf16)
    wvp = pack_rows(np.ascontiguousarray(w_attn[:, 2 * C:3 * C]),
                    1024).astype(bf16)
    wp = shuffle_lhsT(w_proj).astype(bf16)

    # per-batch sink K/V (host-computed, tiny)
    vsink_b, ksink_b = [], []
    for b in range(B):
        k_s = x[b, 0:4] @ w_attn[:, C:2 * C]      # [4, 1024]
        v_s = x[b, 0:4] @ w_attn[:, 2 * C:3 * C]  # [4, 1024]
        vs = np.zeros((4, 1040), np.float32)
        vsr = vs.reshape(4, 16, 65)
        vsr[:, :, 0:64] = v_s.reshape(4, 16, 64)
        vsr[:, :, 64] = 1.0
        vsink_b.append(vs.astype(bf16))
        ks = np.zeros((128, 32), np.float32)
        pos4 = np.arange(4)
        for h in range(NH):
            hp, half = h // 2, h % 2
            kr = rope_rows(k_s[:, h * 64:(h + 1) * 64], pos4)  # [4, 64]
            ks[half * 64:(half + 1) * 64, hp * 4:hp * 4 + 4] = kr.T
        ksink_b.append(ks.astype(bf16))

    in_maps = []
    for core in range(NCORES):
        b, j = core // 4, core % 4
        q0 = j * CH
        # kv columns: [own 512 | halo 256]
        kv_gk = np.full(KV, -1, np.int64)
        kv_gk[0:512] = q0 + np.arange(CH)
        halo = q0 - 256 + np.arange(256)
        kv_gk[512:768] = np.where(halo >= 0, halo, -1)

        xTc = np.zeros((C, KV), np.float32)
        valid = kv_gk >= 0
        xTc[:, valid] = x[b, kv_gk[valid]].T

        cq, sq = cos_sin(q0 + np.arange(CH))
        ck, sk = cos_sin(np.maximum(kv_gk, 0))
        trig = np.concatenate(
            [P2.astype(bf16), cq, sq, ck, sk, ksink_b[b]], axis=1)

        gq = q0 + np.arange(CH)
        mask = np.zeros((128, MTOT), np.float32)
        for c in range(7):
            if c == 6:
                g = np.arange(4)[:, None]                # sink positions
                qq = gq[None, OFF_C[c]:OFF_C[c] + W_C[c]]
                allow = (g <= qq) & (qq - g >= WIN)
                mask[0:4, MOFF[c]:MOFF[c] + W_C[c]] = allow
                continue
            rows = np.arange(128)
            gk = kv_gk[c * 128 + rows]
            qw = gq[OFF_C[c]:OFF_C[c] + W_C[c]]
            real = gk >= 0
            g = np.where(real, gk, 0)[:, None]
            qq = qw[None, :]
            allow = (g <= qq) & (qq - g < WIN) & real[:, None]
            mask[:, MOFF[c]:MOFF[c] + W_C[c]] = allow.astype(np.float32)

        in_maps.append({
            "xad": pack_rows(xTc[:, 0:512], 512).astype(bf16),
            "xbd": pack_rows(xTc[:, 512:768], 256).astype(bf16),
            "wqs": wq, "wks": wk, "wvd": wvp, "wps": wp, "trig": trig,
            "masks": mask.astype(bf16), "vsink": vsink_b[b], "sel2": sel2,
        })
    return in_maps


def kernel(x, w_attn, w_proj):
    from concourse import bass_utils

    x = np.asarray(x, np.float32)
    w_attn = np.asarray(w_attn, np.float32)
    w_proj = np.asarray(w_proj, np.float32)

    if "nc" not in _cache:
        _cache["nc"] = _build_nc()
    nc = _cache["nc"]

    in_maps = _host_inputs(x, w_attn, w_proj)
    res = bass_utils.run_bass_kernel_spmd(nc, in_maps, list(range(NCORES)),
                                          **_cache.get("run_kwargs", {}))
    _cache["last_result"] = res

    y = np.zeros((B, T, C), np.float32)
    for core in range(NCORES):
        b, j = core // 4, core % 4
        y[b, j * CH:(j + 1) * CH, :] = res.results[core]["outT"].T
    return y


# revision 25
# speedup vs baseline: 1.5090x; 1.1567x over previous
"""Trainium2 Bass kernel for CausalSelfAttention with sliding-window + sink mask.

Sharding: 8 cores = (batch 2) x (sequence chunks of 512). Each core computes
QKV (+RoPE) for its 512 queries and a kv range [512 own | 256 halo] = 768
positions (6 chunks of 128); the 4 attention-sink K/V rows are computed on
the host and uploaded as tiny persistent tiles. Banded attention runs in a
scores-transposed [kv, q] layout with per-chunk q-windows, exp on the scalar
engine, multiplicative 0/1 masking split across vector+gpsimd, denominator
via a ones-column in V, per-head-pair normalization inside the loop (PE
broadcast of a 2-row reciprocal), then a preloaded-weight projection that
emits a transposed [C, 512] output the host re-transposes and concatenates.

All matmul operands are bf16 (full-rate PE path, no sub-256 f32r penalty),
accumulation stays f32 in PSUM.
"""

import numpy as np

B, T, C, NH, HD = 2, 2048, 1024, 16, 64
WIN, SINK = 256, 4
CH = 512          # queries per core
KV = 768          # 512 own + 256 halo (sink handled separately)
NCORES = 8
W_C = [384, 384, 256, 128, 128, 256, 512]
OFF_C = [0, 128, 256, 384, 0, 0, 0]
MOFF = np.concatenate([[0], np.cumsum(W_C)]).astype(int)
MTOT = int(MOFF[-1])  # 2048

_cache = {}


def _build_nc():
    import concourse.bacc as bacc
    import concourse.mybir as mybir
    import concourse.tile as tile

    f32 = mybir.dt.float32
    f32r = mybir.dt.float32r
    bf16 = mybir.dt.bfloat16
    AF = mybir.ActivationFunctionType

    nc = bacc.Bacc("TRN2", target_bir_lowering=False, debug=False,
                   num_devices=NCORES)

    # packed layouts (host pre-shuffles) so startup is a handful of DMAs
    xad = nc.dram_tensor("xad", [128, 4096], bf16, kind="ExternalInput").ap()
    xbd = nc.dram_tensor("xbd", [128, 2048], bf16, kind="ExternalInput").ap()
    wqs = nc.dram_tensor("wqs", [C, C], bf16, kind="ExternalInput").ap()
    wks = nc.dram_tensor("wks", [C, C], bf16, kind="ExternalInput").ap()
    wvd = nc.dram_tensor("wvd", [128, 8192], bf16, kind="ExternalInput").ap()
    wps = nc.dram_tensor("wps", [C, C], bf16, kind="ExternalInput").ap()
    # trig blob: p2 | cos_q | sin_q | cos_k | sin_k | ksink
    trig = nc.dram_tensor("trig", [128, 2720], bf16,
                          kind="ExternalInput").ap()
    masks = nc.dram_tensor("masks", [128, MTOT], bf16,
                           kind="ExternalInput").ap()
    vsinkd = nc.dram_tensor("vsink", [4, 1040], bf16,
                            kind="ExternalInput").ap()
    sel2d = nc.dram_tensor("sel2", [2, 128], bf16, kind="ExternalInput").ap()
    outT = nc.dram_tensor("outT", [C, CH], f32, kind="ExternalOutput").ap()

    KSEG = [(0, 512), (512, 256)]  # kv free-dim segments (psum bank limit)
    SCORD = [6, 0, 1, 2, 3, 4, 5]  # sink first: AV accumulation starts full

    with tile.TileContext(nc) as tc:
        with (
            nc.allow_low_precision(reason="bf16 matmul operands throughout"),
            tc.tile_pool(name="pers", bufs=1) as pers,
            tc.tile_pool(name="wsl", bufs=2) as wsl,
            tc.tile_pool(name="big", bufs=8) as big,     # praw/osb rings
            tc.tile_pool(name="wvp", bufs=1) as wvpool,  # packed wv (1 tile)
            tc.tile_pool(name="qk", bufs=2) as qkp,
            tc.tile_pool(name="tmp", bufs=2) as tmp,
            tc.tile_pool(name="yts", bufs=1) as ytsp,
            tc.tile_pool(name="ptp", bufs=10) as ptp,
            tc.tile_pool(name="sm", bufs=4) as smp,
            tc.tile_pool(name="psmm", bufs=2, space="PSUM") as psmm,
            tc.tile_pool(name="pssc", bufs=4, space="PSUM") as pssc,
            tc.tile_pool(name="psyt", bufs=2, space="PSUM") as psyt,
        ):
            # ---------- persistent loads (ordered for fast PE start) ------
            xa_all = pers.tile([128, 4096], bf16, tag="xa")
            nc.sync.dma_start(xa_all[:], xad[:])
            # wq0 DMA is emitted just below; Q0 needs only xa_all+wq0
            xb_all = pers.tile([128, 2048], bf16, tag="xb")
            ttrig = pers.tile([128, 2720], bf16, tag="trig")
            tp2 = ttrig[:, 0:128]
            tcos_q = ttrig[:, 128:640]
            tsin_q = ttrig[:, 640:1152]
            tcos_k = ttrig[:, 1152:1920]
            tsin_k = ttrig[:, 1920:2688]
            tksink = ttrig[:, 2688:2720]
            tsel = []

            def late_loads():
                nc.sync.dma_start(xb_all[:], xbd[:])
                nc.sync.dma_start(ttrig[:], trig[:])
                for half in range(2):
                    t = pers.tile([1, 128], bf16, tag=f"sel{half}")
                    nc.sync.dma_start(t[:], sel2d[half:half + 1, :])
                    tsel.append(t)

            # ---------- qkv + rope (emitted per head-pair) ----------------
            def qkv_rope(hp, wq_pre=None):
                if wq_pre is None:
                    wq_sl = wsl.tile([128, 1024], bf16, tag="wslab",
                                     name=f"wq{hp}")
                    nc.sync.dma_start(wq_sl[:],
                                      wqs[hp * 128:(hp + 1) * 128, :])
                else:
                    wq_sl = wq_pre
                pq = psmm.tile([128, 512], f32, tag="mm", name=f"pq{hp}")
                for kc in range(8):
                    nc.tensor.matmul(
                        pq[:], wq_sl[:, kc * 128:(kc + 1) * 128],
                        xa_all[:, kc * 512:(kc + 1) * 512],
                        start=(kc == 0), stop=(kc == 7),
                    )
                qraw = tmp.tile([128, CH], bf16, tag="qraw", name=f"qraw{hp}")
                nc.scalar.copy(qraw[:], pq[:])

                wk_sl = wsl.tile([128, 1024], bf16, tag="wslab",
                                 name=f"wk{hp}")
                nc.sync.dma_start(wk_sl[:], wks[hp * 128:(hp + 1) * 128, :])
                kraw = tmp.tile([128, KV], bf16, tag="kraw", name=f"kraw{hp}")
                pk = [psmm.tile([128, 512], f32, tag="mm", name=f"pk{hp}_{i}")
                      for i in range(2)]
                for kc in range(8):
                    for si, (s0, sw) in enumerate(KSEG):
                        rhs = (xa_all[:, kc * 512:(kc + 1) * 512] if si == 0
                               else xb_all[:, kc * 256:(kc + 1) * 256])
                        nc.tensor.matmul(
                            pk[si][:, 0:sw], wk_sl[:, kc * 128:(kc + 1) * 128],
                            rhs, start=(kc == 0), stop=(kc == 7),
                        )
                for si, (s0, sw) in enumerate(KSEG):
                    nc.scalar.copy(kraw[:, s0:s0 + sw], pk[si][:, 0:sw])

                # rope: out = raw*cos + (P2@raw)*sin ; all-bf16 muls get DVE 2x
                qT = qkp.tile([128, CH], bf16, tag="qT", name=f"qT{hp}")
                prot = psmm.tile([128, 512], f32, tag="mm", name=f"prot{hp}")
                nc.tensor.matmul(prot[:], tp2[:], qraw[:], start=True,
                                 stop=True)
                t2 = tmp.tile([128, CH], bf16, tag="t2", name=f"t2q{hp}")
                nc.vector.tensor_mul(t2[:], prot[:], tsin_q[:])
                qc = tmp.tile([128, CH], bf16, tag="qc", name=f"qc{hp}")
                nc.vector.tensor_mul(qc[:], qraw[:], tcos_q[:])
                nc.vector.tensor_add(qT[:], qc[:], t2[:])

                kT = qkp.tile([128, KV], bf16, tag="kT", name=f"kT{hp}")
                for si, (s0, sw) in enumerate(KSEG):
                    prk = psmm.tile([128, 512], f32, tag="mm",
                                    name=f"prk{hp}_{si}")
                    nc.tensor.matmul(prk[:, 0:sw], tp2[:],
                                     kraw[:, s0:s0 + sw], start=True,
                                     stop=True)
                    t2k = tmp.tile([128, 512], bf16, tag="t2",
                                   name=f"t2k{hp}_{si}")
                    nc.vector.tensor_mul(t2k[:, 0:sw], prk[:, 0:sw],
                                         tsin_k[:, s0:s0 + sw])
                    kck = tmp.tile([128, 512], bf16, tag="qc",
                                   name=f"kc{hp}_{si}")
                    nc.vector.tensor_mul(kck[:, 0:sw], kraw[:, s0:s0 + sw],
                                         tcos_k[:, s0:s0 + sw])
                    nc.vector.tensor_add(kT[:, s0:s0 + sw], kck[:, 0:sw],
                                         t2k[:, 0:sw])
                return qT, kT

            wq0 = wsl.tile([128, 1024], bf16, tag="wslab", name="wq0")
            nc.sync.dma_start(wq0[:], wqs[0:128, :])
            late_loads()
            qk_state = qkv_rope(0, wq_pre=wq0)

            # ---------- V = xT.T @ wv (6 chunks; sink V preloaded) --------
            wv_all = wvpool.tile([128, 8192], bf16, tag="wvall",
                                 name="wvall")
            nc.sync.dma_start(wv_all[:], wvd[:])
            wvc = [wv_all[:, kc * 1024:(kc + 1) * 1024] for kc in range(8)]
            tvsink = pers.tile([4, 1040], bf16, tag="vsink")
            nc.sync.dma_start(tvsink[:], vsinkd[:])
            tmask = pers.tile([128, MTOT], bf16, tag="mask")
            nc.sync.dma_start(tmask[:], masks[:])

            v_sb = []
            for tt in range(6):
                vt = pers.tile([128, 1040], bf16, tag=f"v{tt}", name=f"v{tt}")
                vr = vt.rearrange("p (h e) -> p h e", e=65)
                pv = [psmm.tile([128, 512], f32, tag="mm", name=f"pv{tt}_{i}")
                      for i in range(2)]
                for kc in range(8):
                    xsl = (xa_all[:, kc * 512 + tt * 128:
                                  kc * 512 + (tt + 1) * 128] if tt < 4 else
                           xb_all[:, kc * 256 + (tt - 4) * 128:
                                  kc * 256 + (tt - 3) * 128])
                    for dh in range(2):
                        nc.tensor.matmul(
                            pv[dh][:], xsl,
                            wvc[kc][:, dh * 512:(dh + 1) * 512],
                            start=(kc == 0), stop=(kc == 7),
                        )
                for dh in range(2):
                    nc.scalar.copy(
                        vr[:, dh * 8:(dh + 1) * 8, 0:64],
                        pv[dh][:].rearrange("p (h e) -> p h e", e=64),
                    )
                nc.vector.memset(vr[:, :, 64:65], 1.0)
                v_sb.append(vt)

            # preload projection weights during the loop
            wp_sb = []
            for cc in range(8):
                t = pers.tile([128, 1024], bf16, tag=f"wp{cc}",
                              name=f"wp{cc}")
                nc.sync.dma_start(t[:], wps[cc * 128:(cc + 1) * 128, :])
                wp_sb.append(t)

            # ---------- scores + exp + mask -------------------------------
            # psum pairing: (c2,c3) and (c4,c5) share a tile/activation
            PAIR = {2: (2, 0), 3: (2, 256), 4: (4, 0), 5: (4, 128)}

            def sc_block(hp, qT, kT):
                pts = {}
                for half in range(2):
                    dsl = slice(half * 64, half * 64 + 64)
                    scs = {}
                    for c in SCORD:
                        w, off = W_C[c], OFF_C[c]
                        base, bo = PAIR.get(c, (c, 0))
                        if base not in scs:
                            scs[base] = pssc.tile(
                                [128, 512], f32, tag="sc",
                                name=f"sc{hp}_{base}_{half}")
                        lhsT = (tksink[dsl, hp * 4:hp * 4 + 4] if c == 6 else
                                kT[dsl, c * 128:(c + 1) * 128])
                        rows = 4 if c == 6 else 128
                        nc.tensor.matmul(
                            scs[base][0:rows, bo:bo + w], lhsT,
                            qT[dsl, off:off + w], start=True, stop=True,
                        )
                    for base, tw, rows in ((6, 512, 4), (0, 384, 128),
                                           (1, 384, 128), (2, 384, 128),
                                           (4, 384, 128)):
                        praw = big.tile([128, 512], bf16, tag="big",
                                        name=f"praw{hp}_{base}_{half}")
                        nc.scalar.activation(praw[0:rows, 0:tw],
                                             scs[base][0:rows, 0:tw],
                                             AF.Exp, scale=0.125)
                        pt = ptp.tile([128, 512], bf16, tag="pt",
                                      name=f"pt{hp}_{base}_{half}")
                        # gpsimd takes a minority share (it runs ~2ns/elem);
                        # DVE bf16-2x handles the rest without stalling AV
                        eng = (nc.gpsimd if half == 1 and base in (0, 1)
                               else nc.vector)
                        eng.tensor_mul(
                            pt[0:rows, 0:tw], praw[0:rows, 0:tw],
                            tmask[0:rows, MOFF[base]:MOFF[base] + tw],
                        )
                        pts[(base, half)] = pt
                return pts

            # ---------- AV + per-hp normalization -------------------------
            def av_block(hp, pts):
                yt_pair = []
                for half in range(2):
                    h = hp * 2 + half
                    yt = psyt.tile([65, 512], f32, tag="yt",
                                   name=f"yt{hp}_{half}")
                    for ci, c in enumerate(SCORD):
                        w, off = W_C[c], OFF_C[c]
                        base, bo = PAIR.get(c, (c, 0))
                        lhsT = (tvsink[:, h * 65:(h + 1) * 65] if c == 6 else
                                v_sb[c][:, h * 65:(h + 1) * 65])
                        nc.tensor.matmul(
                            yt[:, off:off + w], lhsT,
                            pts[(base, half)][0:(4 if c == 6 else 128),
                                              bo:bo + w],
                            start=(ci == 0), stop=(ci == 6),
                        )
                    yt_pair.append(yt)
                # denominators gathered; reciprocal deferred to apply_norm so
                # the DVE queue serves the next block's masks first
                dt1 = smp.tile([1, 1024], f32, tag="dt", name=f"dt{hp}")
                for half in range(2):
                    nc.scalar.copy(dt1[0:1, half * 512:(half + 1) * 512],
                                   yt_pair[half][64:65, :])
                ytu = ytsp.tile([128, CH], bf16, tag=f"ytu{hp}",
                                name=f"ytu{hp}")
                nc.scalar.copy(ytu[0:64, :], yt_pair[0][0:64, :])
                nc.scalar.copy(ytu[64:128, :], yt_pair[1][0:64, :])
                return ytu, dt1

            def apply_norm(hp, ytu, dt1):
                r1 = smp.tile([1, 1024], f32, tag="r2", name=f"r{hp}")
                nc.vector.reciprocal_approx_fast(r1[:], dt1[:])
                r1b = smp.tile([1, 1024], bf16, tag="r2b", name=f"rb{hp}")
                nc.scalar.copy(r1b[:], r1[:])
                prb = psmm.tile([128, 512], f32, tag="mm", name=f"prb{hp}")
                for half in range(2):
                    nc.tensor.matmul(
                        prb[:], tsel[half][:],
                        r1b[0:1, half * 512:(half + 1) * 512],
                        start=(half == 0), stop=(half == 1))
                nc.vector.tensor_mul(ytu[:], ytu[:], prb[:])
                return ytu

            # ---------- software-pipelined head-pair loop -----------------
            yts = []
            pend = None
            for hp in range(8):
                pts = sc_block(hp, *qk_state)
                if hp < 7:
                    qk_state = qkv_rope(hp + 1)
                ytu, rh = av_block(hp, pts)
                if pend is not None:
                    yts.append(apply_norm(*pend))
                pend = (hp, ytu, rh)
            yts.append(apply_norm(*pend))

            # ---------- projection (weights preloaded) --------------------
            for cc in range(8):
                po = psmm.tile([128, 512], f32, tag="mm", name=f"po{cc}")
                for hp in range(8):
                    nc.tensor.matmul(
                        po[:], wp_sb[cc][:, hp * 128:(hp + 1) * 128],
                        yts[hp][:],
                        start=(hp == 0), stop=(hp == 7),
                    )
                osb = big.tile([128, 512], f32, tag="osb", name=f"osb{cc}")
                nc.scalar.copy(osb[:], po[:])
                nc.sync.dma_start(outT[cc * 128:(cc + 1) * 128, :], osb[:])

    nc.compile()
    return nc


def _host_inputs(x, w_attn, w_proj):
    """Build the 8 per-core input maps."""
    import ml_dtypes
    bf16 = ml_dtypes.bfloat16

    inv_freq = 1.0 / (10000.0 ** (np.arange(0, HD, 2, dtype=np.float32) / HD))
    iff = np.concatenate([inv_freq, inv_freq])  # [64]

    def cos_sin(pos):
        ang = pos[None, :].astype(np.float32) * iff[:, None]
        c = np.concatenate([np.cos(ang), np.cos(ang)], 0)
        s = np.concatenate([np.sin(ang), np.sin(ang)], 0)
        return (np.ascontiguousarray(c).astype(bf16),
                np.ascontiguousarray(s).astype(bf16))

    def rope_rows(v, pos):
        # v [n, 64] at positions pos -> rope'd [n, 64]
        ang = pos[:, None].astype(np.float32) * iff[None, :]
        cos, sin = np.cos(ang), np.sin(ang)
        rot = np.concatenate([-v[:, 32:], v[:, :32]], axis=1)
        return v * cos + rot * sin

    P2 = np.zeros((128, 128), np.float32)
    for blk in range(2):
        o = blk * 64
        for d in range(32):
            P2[o + d + 32, o + d] = -1.0
            P2[o + d, o + d + 32] = 1.0

    sel2 = np.zeros((2, 128), np.float32)
    sel2[0, 0:64] = 1.0
    sel2[1, 64:128] = 1.0
    sel2 = sel2.astype(bf16)

    def shuffle_lhsT(w):
        return np.ascontiguousarray(
            w.reshape(8, 128, 8, 128).transpose(2, 1, 0, 3).reshape(C, C)
        )

    def pack_rows(w, colw):
        # [1024, colw] -> [128, 8*colw]: out[p, kc*colw+t] = w[kc*128+p, t]
        return np.ascontiguousarray(
            w.reshape(8, 128, colw).transpose(1, 0, 2).reshape(128, 8 * colw)
        )

    wq = shuffle_lhsT(w_attn[:, 0:C]).astype(bf16)
    wk = shuffle_lhsT(w_attn[:, C:2 * C]).astype(bf16)
    wvp = pack_rows(np.ascontiguousarray(w_attn[:, 2 * C:3 * C]),
                    1024).astype(bf16)
    wp = shuffle_lhsT(w_proj).astype(bf16)

    # per-batch sink K/V (host-computed, tiny)
    vsink_b, ksink_b = [], []
    for b in range(B):
        k_s = x[b, 0:4] @ w_attn[:, C:2 * C]      # [4, 1024]
        v_s = x[b, 0:4] @ w_attn[:, 2 * C:3 * C]  # [4, 1024]
        vs = np.zeros((4, 1040), np.float32)
        vsr = vs.reshape(4, 16, 65)
        vsr[:, :, 0:64] = v_s.reshape(4, 16, 64)
        vsr[:, :, 64] = 1.0
        vsink_b.append(vs.astype(bf16))
        ks = np.zeros((128, 32), np.float32)
        pos4 = np.arange(4)
        for h in range(NH):
            hp, half = h // 2, h % 2
            kr = rope_rows(k_s[:, h * 64:(h + 1) * 64], pos4)  # [4, 64]
            ks[half * 64:(half + 1) * 64, hp * 4:hp * 4 + 4] = kr.T
        ksink_b.append(ks.astype(bf16))

    in_maps = []
    for core in range(NCORES):
        b, j = core // 4, core % 4
        q0 = j * CH
        # kv columns: [own 512 | halo 256]
        kv_gk = np.full(KV, -1, np.int64)
        kv_gk[0:512] = q0 + np.arange(CH)
        halo = q0 - 256 + np.arange(256)
        kv_gk[512:768] = np.where(halo >= 0, halo, -1)

        xTc = np.zeros((C, KV), np.float32)
        valid = kv_gk >= 0
        xTc[:, valid] = x[b, kv_gk[valid]].T

        cq, sq = cos_sin(q0 + np.arange(CH))
        ck, sk = cos_sin(np.maximum(kv_gk, 0))
        trig = np.concatenate(
            [P2.astype(bf16), cq, sq, ck, sk, ksink_b[b]], axis=1)

        gq = q0 + np.arange(CH)
        mask = np.zeros((128, MTOT), np.float32)
        for c in range(7):
            if c == 6:
                g = np.arange(4)[:, None]                # sink positions
                qq = gq[None, OFF_C[c]:OFF_C[c] + W_C[c]]
                allow = (g <= qq) & (qq - g >= WIN)
                mask[0:4, MOFF[c]:MOFF[c] + W_C[c]] = allow
                continue
            rows = np.arange(128)
            gk = kv_gk[c * 128 + rows]
            qw = gq[OFF_C[c]:OFF_C[c] + W_C[c]]
            real = gk >= 0
            g = np.where(real, gk, 0)[:, None]
            qq = qw[None, :]
            allow = (g <= qq) & (qq - g < WIN) & real[:, None]
            mask[:, MOFF[c]:MOFF[c] + W_C[c]] = allow.astype(np.float32)

        in_maps.append({
            "xad": pack_rows(xTc[:, 0:512], 512).astype(bf16),
            "xbd": pack_rows(xTc[:, 512:768], 256).astype(bf16),
            "wqs": wq, "wks": wk, "wvd": wvp, "wps": wp, "trig": trig,
            "masks": mask.astype(bf16), "vsink": vsink_b[b], "sel2": sel2,
        })
    return in_maps


def kernel(x, w_attn, w_proj):
    from concourse import bass_utils

    x = np.asarray(x, np.float32)
    w_attn = np.asarray(w_attn, np.float32)
    w_proj = np.asarray(w_proj, np.float32)

    if "nc" not in _cache:
        _cache["nc"] = _build_nc()
    nc = _cache["nc"]

    in_maps = _host_inputs(x, w_attn, w_proj)
    res = bass_utils.run_bass_kernel_spmd(nc, in_maps, list(range(NCORES)),
                                          **_cache.get("run_kwargs", {}))
    _cache["last_result"] = res

    y = np.zeros((B, T, C), np.float32)
    for core in range(NCORES):
        b, j = core // 4, core % 4
        y[b, j * CH:(j + 1) * CH, :] = res.results[core]["outT"].T
    return y


# revision 26
# speedup vs baseline: 1.5100x; 1.0007x over previous
"""Trainium2 Bass kernel for CausalSelfAttention with sliding-window + sink mask.

Sharding: 8 cores = (batch 2) x (sequence chunks of 512). Each core computes
QKV (+RoPE) for its 512 queries and a kv range [512 own | 256 halo] = 768
positions (6 chunks of 128); the 4 attention-sink K/V rows are computed on
the host and uploaded as tiny persistent tiles. Banded attention runs in a
scores-transposed [kv, q] layout with per-chunk q-windows, exp on the scalar
engine, multiplicative 0/1 masking split across vector+gpsimd, denominator
via a ones-column in V, per-head-pair normalization inside the loop (PE
broadcast of a 2-row reciprocal), then a preloaded-weight projection that
emits a transposed [C, 512] output the host re-transposes and concatenates.

All matmul operands are bf16 (full-rate PE path, no sub-256 f32r penalty),
accumulation stays f32 in PSUM.
"""

import numpy as np

B, T, C, NH, HD = 2, 2048, 1024, 16, 64
WIN, SINK = 256, 4
CH = 512          # queries per core
KV = 768          # 512 own + 256 halo (sink handled separately)
NCORES = 8
W_C = [384, 384, 256, 128, 128, 256, 512]
OFF_C = [0, 128, 256, 384, 0, 0, 0]
MOFF = np.concatenate([[0], np.cumsum(W_C)]).astype(int)
MTOT = int(MOFF[-1])  # 2048

_cache = {}


def _build_nc():
    import concourse.bacc as bacc
    import concourse.mybir as mybir
    import concourse.tile as tile

    f32 = mybir.dt.float32
    f32r = mybir.dt.float32r
    bf16 = mybir.dt.bfloat16
    AF = mybir.ActivationFunctionType

    nc = bacc.Bacc("TRN2", target_bir_lowering=False, debug=False,
                   num_devices=NCORES)

    # packed layouts (host pre-shuffles) so startup is a handful of DMAs
    xad = nc.dram_tensor("xad", [128, 4096], bf16, kind="ExternalInput").ap()
    xbd = nc.dram_tensor("xbd", [128, 2048], bf16, kind="ExternalInput").ap()
    wqs = nc.dram_tensor("wqs", [C, C], bf16, kind="ExternalInput").ap()
    wks = nc.dram_tensor("wks", [C, C], bf16, kind="ExternalInput").ap()
    wvd = nc.dram_tensor("wvd", [128, 8192], bf16, kind="ExternalInput").ap()
    wps = nc.dram_tensor("wps", [C, C], bf16, kind="ExternalInput").ap()
    # trig blob: p2 | cos_q | sin_q | cos_k | sin_k | ksink
    trig = nc.dram_tensor("trig", [128, 2720], bf16,
                          kind="ExternalInput").ap()
    masks = nc.dram_tensor("masks", [128, MTOT], bf16,
                           kind="ExternalInput").ap()
    vsinkd = nc.dram_tensor("vsink", [4, 1040], bf16,
                            kind="ExternalInput").ap()
    sel2d = nc.dram_tensor("sel2", [2, 128], bf16, kind="ExternalInput").ap()
    outT = nc.dram_tensor("outT", [C, CH], f32, kind="ExternalOutput").ap()

    KSEG = [(0, 512), (512, 256)]  # kv free-dim segments (psum bank limit)
    SCORD = [6, 0, 1, 2, 3, 4, 5]  # sink first: AV accumulation starts full

    with tile.TileContext(nc) as tc:
        with (
            nc.allow_low_precision(reason="bf16 matmul operands throughout"),
            tc.tile_pool(name="pers", bufs=1) as pers,
            tc.tile_pool(name="wsl", bufs=2) as wsl,
            tc.tile_pool(name="big", bufs=8) as big,     # praw/osb rings
            tc.tile_pool(name="wvp", bufs=1) as wvpool,  # packed wv (1 tile)
            tc.tile_pool(name="qk", bufs=2) as qkp,
            tc.tile_pool(name="tmp", bufs=2) as tmp,
            tc.tile_pool(name="yts", bufs=1) as ytsp,
            tc.tile_pool(name="ptp", bufs=10) as ptp,
            tc.tile_pool(name="sm", bufs=4) as smp,
            tc.tile_pool(name="psmm", bufs=2, space="PSUM") as psmm,
            tc.tile_pool(name="pssc", bufs=4, space="PSUM") as pssc,
            tc.tile_pool(name="psyt", bufs=2, space="PSUM") as psyt,
        ):
            # ---------- persistent loads (ordered for fast PE start) ------
            xa_all = pers.tile([128, 4096], bf16, tag="xa")
            nc.sync.dma_start(xa_all[:], xad[:])
            # wq0 DMA is emitted just below; Q0 needs only xa_all+wq0
            xb_all = pers.tile([128, 2048], bf16, tag="xb")
            ttrig = pers.tile([128, 2720], bf16, tag="trig")
            tp2 = ttrig[:, 0:128]
            tcos_q = ttrig[:, 128:640]
            tsin_q = ttrig[:, 640:1152]
            tcos_k = ttrig[:, 1152:1920]
            tsin_k = ttrig[:, 1920:2688]
            tksink = ttrig[:, 2688:2720]
            tsel = []

            def late_loads():
                nc.sync.dma_start(xb_all[:], xbd[:])
                nc.sync.dma_start(ttrig[:], trig[:])
                for half in range(2):
                    t = pers.tile([1, 128], bf16, tag=f"sel{half}")
                    nc.sync.dma_start(t[:], sel2d[half:half + 1, :])
                    tsel.append(t)

            # ---------- qkv + rope (emitted per head-pair) ----------------
            def qkv_rope(hp, wq_pre=None):
                if wq_pre is None:
                    wq_sl = wsl.tile([128, 1024], bf16, tag="wslab",
                                     name=f"wq{hp}")
                    nc.sync.dma_start(wq_sl[:],
                                      wqs[hp * 128:(hp + 1) * 128, :])
                else:
                    wq_sl = wq_pre
                pq = psmm.tile([128, 512], f32, tag="mm", name=f"pq{hp}")
                for kc in range(8):
                    nc.tensor.matmul(
                        pq[:], wq_sl[:, kc * 128:(kc + 1) * 128],
                        xa_all[:, kc * 512:(kc + 1) * 512],
                        start=(kc == 0), stop=(kc == 7),
                    )
                qraw = tmp.tile([128, CH], bf16, tag="qraw", name=f"qraw{hp}")
                nc.scalar.copy(qraw[:], pq[:])

                wk_sl = wsl.tile([128, 1024], bf16, tag="wslab",
                                 name=f"wk{hp}")
                nc.sync.dma_start(wk_sl[:], wks[hp * 128:(hp + 1) * 128, :])
                kraw = tmp.tile([128, KV], bf16, tag="kraw", name=f"kraw{hp}")
                pk = [psmm.tile([128, 512], f32, tag="mm", name=f"pk{hp}_{i}")
                      for i in range(2)]
                for kc in range(8):
                    for si, (s0, sw) in enumerate(KSEG):
                        rhs = (xa_all[:, kc * 512:(kc + 1) * 512] if si == 0
                               else xb_all[:, kc * 256:(kc + 1) * 256])
                        nc.tensor.matmul(
                            pk[si][:, 0:sw], wk_sl[:, kc * 128:(kc + 1) * 128],
                            rhs, start=(kc == 0), stop=(kc == 7),
                        )
                for si, (s0, sw) in enumerate(KSEG):
                    nc.scalar.copy(kraw[:, s0:s0 + sw], pk[si][:, 0:sw])

                # rope: out = raw*cos + (P2@raw)*sin ; all-bf16 muls get DVE 2x
                qT = qkp.tile([128, CH], bf16, tag="qT", name=f"qT{hp}")
                prot = psmm.tile([128, 512], f32, tag="mm", name=f"prot{hp}")
                nc.tensor.matmul(prot[:], tp2[:], qraw[:], start=True,
                                 stop=True)
                t2 = tmp.tile([128, CH], bf16, tag="t2", name=f"t2q{hp}")
                nc.vector.tensor_mul(t2[:], prot[:], tsin_q[:])
                qc = tmp.tile([128, CH], bf16, tag="qc", name=f"qc{hp}")
                nc.vector.tensor_mul(qc[:], qraw[:], tcos_q[:])
                nc.vector.tensor_add(qT[:], qc[:], t2[:])

                kT = qkp.tile([128, KV], bf16, tag="kT", name=f"kT{hp}")
                for si, (s0, sw) in enumerate(KSEG):
                    prk = psmm.tile([128, 512], f32, tag="mm",
                                    name=f"prk{hp}_{si}")
                    nc.tensor.matmul(prk[:, 0:sw], tp2[:],
                                     kraw[:, s0:s0 + sw], start=True,
                                     stop=True)
                    t2k = tmp.tile([128, 512], bf16, tag="t2",
                                   name=f"t2k{hp}_{si}")
                    nc.vector.tensor_mul(t2k[:, 0:sw], prk[:, 0:sw],
                                         tsin_k[:, s0:s0 + sw])
                    kck = tmp.tile([128, 512], bf16, tag="qc",
                                   name=f"kc{hp}_{si}")
                    nc.vector.tensor_mul(kck[:, 0:sw], kraw[:, s0:s0 + sw],
                                         tcos_k[:, s0:s0 + sw])
                    nc.vector.tensor_add(kT[:, s0:s0 + sw], kck[:, 0:sw],
                                         t2k[:, 0:sw])
                return qT, kT

            wq0 = wsl.tile([128, 1024], bf16, tag="wslab", name="wq0")
            nc.sync.dma_start(wq0[:], wqs[0:128, :])
            late_loads()
            qk_state = qkv_rope(0, wq_pre=wq0)

            # ---------- V = xT.T @ wv (6 chunks; sink V preloaded) --------
            wv_all = wvpool.tile([128, 8192], bf16, tag="wvall",
                                 name="wvall")
            nc.sync.dma_start(wv_all[:], wvd[:])
            wvc = [wv_all[:, kc * 1024:(kc + 1) * 1024] for kc in range(8)]
            tvsink = pers.tile([4, 1040], bf16, tag="vsink")
            nc.sync.dma_start(tvsink[:], vsinkd[:])
            tmask = pers.tile([128, MTOT], bf16, tag="mask")
            nc.sync.dma_start(tmask[:], masks[:])

            v_sb = []
            for tt in range(6):
                vt = pers.tile([128, 1040], bf16, tag=f"v{tt}", name=f"v{tt}")
                vr = vt.rearrange("p (h e) -> p h e", e=65)
                pv = [psmm.tile([128, 512], f32, tag="mm", name=f"pv{tt}_{i}")
                      for i in range(2)]
                for kc in range(8):
                    xsl = (xa_all[:, kc * 512 + tt * 128:
                                  kc * 512 + (tt + 1) * 128] if tt < 4 else
                           xb_all[:, kc * 256 + (tt - 4) * 128:
                                  kc * 256 + (tt - 3) * 128])
                    for dh in range(2):
                        nc.tensor.matmul(
                            pv[dh][:], xsl,
                            wvc[kc][:, dh * 512:(dh + 1) * 512],
                            start=(kc == 0), stop=(kc == 7),
                        )
                for dh in range(2):
                    nc.scalar.copy(
                        vr[:, dh * 8:(dh + 1) * 8, 0:64],
                        pv[dh][:].rearrange("p (h e) -> p h e", e=64),
                    )
                nc.vector.memset(vr[:, :, 64:65], 1.0)
                v_sb.append(vt)

            # preload projection weights during the loop
            wp_sb = []
            for cc in range(8):
                t = pers.tile([128, 1024], bf16, tag=f"wp{cc}",
                              name=f"wp{cc}")
                nc.sync.dma_start(t[:], wps[cc * 128:(cc + 1) * 128, :])
                wp_sb.append(t)

            # ---------- scores + exp + mask -------------------------------
            # psum pairing: (c2,c3) and (c4,c5) share a tile/activation
            PAIR = {2: (2, 0), 3: (2, 256), 4: (4, 0), 5: (4, 128)}

            def sc_block(hp, qT, kT):
                pts = {}
                for half in range(2):
                    dsl = slice(half * 64, half * 64 + 64)
                    scs = {}
                    for c in SCORD:
                        w, off = W_C[c], OFF_C[c]
                        base, bo = PAIR.get(c, (c, 0))
                        if base not in scs:
                            scs[base] = pssc.tile(
                                [128, 512], f32, tag="sc",
                                name=f"sc{hp}_{base}_{half}")
                        lhsT = (tksink[dsl, hp * 4:hp * 4 + 4] if c == 6 else
                                kT[dsl, c * 128:(c + 1) * 128])
                        rows = 4 if c == 6 else 128
                        nc.tensor.matmul(
                            scs[base][0:rows, bo:bo + w], lhsT,
                            qT[dsl, off:off + w], start=True, stop=True,
                        )
                    for base, tw, rows in ((6, 512, 4), (0, 384, 128),
                                           (1, 384, 128), (2, 384, 128),
                                           (4, 384, 128)):
                        praw = big.tile([128, 512], bf16, tag="big",
                                        name=f"praw{hp}_{base}_{half}")
                        nc.scalar.activation(praw[0:rows, 0:tw],
                                             scs[base][0:rows, 0:tw],
                                             AF.Exp, scale=0.125)
                        pt = ptp.tile([128, 512], bf16, tag="pt",
                                      name=f"pt{hp}_{base}_{half}")
                        eng = nc.vector
                        eng.tensor_mul(
                            pt[0:rows, 0:tw], praw[0:rows, 0:tw],
                            tmask[0:rows, MOFF[base]:MOFF[base] + tw],
                        )
                        pts[(base, half)] = pt
                return pts

            # ---------- AV + per-hp normalization -------------------------
            def av_block(hp, pts):
                yt_pair = []
                for half in range(2):
                    h = hp * 2 + half
                    yt = psyt.tile([65, 512], f32, tag="yt",
                                   name=f"yt{hp}_{half}")
                    for ci, c in enumerate(SCORD):
                        w, off = W_C[c], OFF_C[c]
                        base, bo = PAIR.get(c, (c, 0))
                        lhsT = (tvsink[:, h * 65:(h + 1) * 65] if c == 6 else
                                v_sb[c][:, h * 65:(h + 1) * 65])
                        nc.tensor.matmul(
                            yt[:, off:off + w], lhsT,
                            pts[(base, half)][0:(4 if c == 6 else 128),
                                              bo:bo + w],
                            start=(ci == 0), stop=(ci == 6),
                        )
                    yt_pair.append(yt)
                # denominators gathered; reciprocal deferred to apply_norm so
                # the DVE queue serves the next block's masks first
                dt1 = smp.tile([1, 1024], f32, tag="dt", name=f"dt{hp}")
                for half in range(2):
                    nc.scalar.copy(dt1[0:1, half * 512:(half + 1) * 512],
                                   yt_pair[half][64:65, :])
                ytu = ytsp.tile([128, CH], bf16, tag=f"ytu{hp}",
                                name=f"ytu{hp}")
                nc.scalar.copy(ytu[0:64, :], yt_pair[0][0:64, :])
                nc.scalar.copy(ytu[64:128, :], yt_pair[1][0:64, :])
                return ytu, dt1

            def apply_norm(hp, ytu, dt1):
                r1 = smp.tile([1, 1024], f32, tag="r2", name=f"r{hp}")
                nc.vector.reciprocal_approx_fast(r1[:], dt1[:])
                r1b = smp.tile([1, 1024], bf16, tag="r2b", name=f"rb{hp}")
                nc.scalar.copy(r1b[:], r1[:])
                prb = psmm.tile([128, 512], f32, tag="mm", name=f"prb{hp}")
                for half in range(2):
                    nc.tensor.matmul(
                        prb[:], tsel[half][:],
                        r1b[0:1, half * 512:(half + 1) * 512],
                        start=(half == 0), stop=(half == 1))
                nc.vector.tensor_mul(ytu[:], ytu[:], prb[:])
                return ytu

            # ---------- software-pipelined head-pair loop -----------------
            yts = []
            pend = None
            for hp in range(8):
                pts = sc_block(hp, *qk_state)
                if hp < 7:
                    qk_state = qkv_rope(hp + 1)
                ytu, rh = av_block(hp, pts)
                if pend is not None:
                    yts.append(apply_norm(*pend))
                pend = (hp, ytu, rh)
            yts.append(apply_norm(*pend))

            # ---------- projection (weights preloaded) --------------------
            for cc in range(8):
                po = psmm.tile([128, 512], f32, tag="mm", name=f"po{cc}")
                for hp in range(8):
                    nc.tensor.matmul(
                        po[:], wp_sb[cc][:, hp * 128:(hp + 1) * 128],
                        yts[hp][:],
                        start=(hp == 0), stop=(hp == 7),
                    )
                osb = big.tile([128, 512], f32, tag="osb", name=f"osb{cc}")
                nc.scalar.copy(osb[:], po[:])
                nc.sync.dma_start(outT[cc * 128:(cc + 1) * 128, :], osb[:])

    nc.compile()
    return nc


def _host_inputs(x, w_attn, w_proj):
    """Build the 8 per-core input maps."""
    import ml_dtypes
    bf16 = ml_dtypes.bfloat16

    inv_freq = 1.0 / (10000.0 ** (np.arange(0, HD, 2, dtype=np.float32) / HD))
    iff = np.concatenate([inv_freq, inv_freq])  # [64]

    def cos_sin(pos):
        ang = pos[None, :].astype(np.float32) * iff[:, None]
        c = np.concatenate([np.cos(ang), np.cos(ang)], 0)
        s = np.concatenate([np.sin(ang), np.sin(ang)], 0)
        return (np.ascontiguousarray(c).astype(bf16),
                np.ascontiguousarray(s).astype(bf16))

    def rope_rows(v, pos):
        # v [n, 64] at positions pos -> rope'd [n, 64]
        ang = pos[:, None].astype(np.float32) * iff[None, :]
        cos, sin = np.cos(ang), np.sin(ang)
        rot = np.concatenate([-v[:, 32:], v[:, :32]], axis=1)
        return v * cos + rot * sin

    P2 = np.zeros((128, 128), np.float32)
    for blk in range(2):
        o = blk * 64
        for d in range(32):
            P2[o + d + 32, o + d] = -1.0
            P2[o + d, o + d + 32] = 1.0

    sel2 = np.zeros((2, 128), np.float32)
    sel2[0, 0:64] = 1.0
    sel2[1, 64:128] = 1.0
    sel2 = sel2.astype(bf16)

    def shuffle_lhsT(w):
        return np.ascontiguousarray(
            w.reshape(8, 128, 8, 128).transpose(2, 1, 0, 3).reshape(C, C)
        )

    def pack_rows(w, colw):
        # [1024, colw] -> [128, 8*colw]: out[p, kc*colw+t] = w[kc*128+p, t]
        return np.ascontiguousarray(
            w.reshape(8, 128, colw).transpose(1, 0, 2).reshape(128, 8 * colw)
        )

    wq = shuffle_lhsT(w_attn[:, 0:C]).astype(bf16)
    wk = shuffle_lhsT(w_attn[:, C:2 * C]).astype(bf16)
    wvp = pack_rows(np.ascontiguousarray(w_attn[:, 2 * C:3 * C]),
                    1024).astype(bf16)
    wp = shuffle_lhsT(w_proj).astype(bf16)

    # per-batch sink K/V (host-computed, tiny)
    vsink_b, ksink_b = [], []
    for b in range(B):
        k_s = x[b, 0:4] @ w_attn[:, C:2 * C]      # [4, 1024]
        v_s = x[b, 0:4] @ w_attn[:, 2 * C:3 * C]  # [4, 1024]
        vs = np.zeros((4, 1040), np.float32)
        vsr = vs.reshape(4, 16, 65)
        vsr[:, :, 0:64] = v_s.reshape(4, 16, 64)
        vsr[:, :, 64] = 1.0
        vsink_b.append(vs.astype(bf16))
        ks = np.zeros((128, 32), np.float32)
        pos4 = np.arange(4)
        for h in range(NH):
            hp, half = h // 2, h % 2
            kr = rope_rows(k_s[:, h * 64:(h + 1) * 64], pos4)  # [4, 64]
            ks[half * 64:(half + 1) * 64, hp * 4:hp * 4 + 4] = kr.T
        ksink_b.append(ks.astype(bf16))

    in_maps = []
    for core in range(NCORES):
        b, j = core // 4, core % 4
        q0 = j * CH
        # kv columns: [own 512 | halo 256]
        kv_gk = np.full(KV, -1, np.int64)
        kv_gk[0:512] = q0 + np.arange(CH)
        halo = q0 - 256 + np.arange(256)
        kv_gk[512:768] = np.where(halo >= 0, halo, -1)

        xTc = np.zeros((C, KV), np.float32)
        valid = kv_gk >= 0
        xTc[:, valid] = x[b, kv_gk[valid]].T

        cq, sq = cos_sin(q0 + np.arange(CH))
        ck, sk = cos_sin(np.maximum(kv_gk, 0))
        trig = np.concatenate(
            [P2.astype(bf16), cq, sq, ck, sk, ksink_b[b]], axis=1)

        gq = q0 + np.arange(CH)
        mask = np.zeros((128, MTOT), np.float32)
        for c in range(7):
            if c == 6:
                g = np.arange(4)[:, None]                # sink positions
                qq = gq[None, OFF_C[c]:OFF_C[c] + W_C[c]]
                allow = (g <= qq) & (qq - g >= WIN)
                mask[0:4, MOFF[c]:MOFF[c] + W_C[c]] = allow
                continue
            rows = np.arange(128)
            gk = kv_gk[c * 128 + rows]
            qw = gq[OFF_C[c]:OFF_C[c] + W_C[c]]
            real = gk >= 0
            g = np.where(real, gk, 0)[:, None]
            qq = qw[None, :]
            allow = (g <= qq) & (qq - g < WIN) & real[:, None]
            mask[:, MOFF[c]:MOFF[c] + W_C[c]] = allow.astype(np.float32)

        in_maps.append({
            "xad": pack_rows(xTc[:, 0:512], 512).astype(bf16),
            "xbd": pack_rows(xTc[:, 512:768], 256).astype(bf16),
            "wqs": wq, "wks": wk, "wvd": wvp, "wps": wp, "trig": trig,
            "masks": mask.astype(bf16), "vsink": vsink_b[b], "sel2": sel2,
        })
    return in_maps


def kernel(x, w_attn, w_proj):
    from concourse import bass_utils

    x = np.asarray(x, np.float32)
    w_attn = np.asarray(w_attn, np.float32)
    w_proj = np.asarray(w_proj, np.float32)

    if "nc" not in _cache:
        _cache["nc"] = _build_nc()
    nc = _cache["nc"]

    in_maps = _host_inputs(x, w_attn, w_proj)
    res = bass_utils.run_bass_kernel_spmd(nc, in_maps, list(range(NCORES)),
                                          **_cache.get("run_kwargs", {}))
    _cache["last_result"] = res

    y = np.zeros((B, T, C), np.float32)
    for core in range(NCORES):
        b, j = core // 4, core % 4
        y[b, j * CH:(j + 1) * CH, :] = res.results[core]["outT"].T
    return y
